# revision 15
# baseline (speedup 1.0000x reference)
"""
MinibatchDiscrimination kernel for 8x TRN2 NeuronCores (Bass/Tile).

Math:  x = inputs @ T  -> [B, K, D] with B=512, K=100, D=5
       out[a,k] = sum_b exp(-sum_d |x[a,k,d]-x[b,k,d]|)

v4 strategy (v2 = 49.8us, v3 experiments showed per-row ACT exp overhead
and batched-multiply chains were the limiters):

  Pair coverage (as v2): core c owns global rows a = 64c+j (j=0..63) and
  window delta = 1..256 (partners b = a+delta mod 512). Deltas 1..255
  cover each unordered pair once; delta=256 pairs appear from both
  endpoints, and each endpoint keeps its own copy in its row sum while
  the cross path scatters only delta=1..255, so no correction columns
  are needed. The self term exp(0)=1 is added on the host.

  Identity: |u-v| = 2*relu(u-v) - u + v  =>  dist = 2R' - S_b + S_a with
  R' = sum_d relu(x_b - x_a), S = sum_d x. PSUM accumulates
  (R' - S_b/2) per row (4 ones-matmuls + one -I matmul with the halved
  S16 window), so one BIAS-FREE activation computes
      P = exp(-2*psum) = exp(S_b - 2R') = exp(S_a - dist)
  batched over FOUR rows at a time ([128,1024] spanning 2 psum banks,
  260ns/row vs v2's 585ns/row exp+accum). P can reach exp(+17) so P8 is
  stored in BF16 (8-bit exponent - no overflow; 0.4% mantissa error is
  well inside tolerance). A single DVE tensor_scalar rider per row then
  applies the row factor and the row sum in one 4x-mode op:
      dump = P * exp(-S_a)   (G scalar, f32),  accum_out = sum(dump).

  Per row j:
    Pool: relu chunk 0 (ready earliest from the projection), 213ns
    DVE : relu chunks 1,2,3 (fp16 4x mode, 127ns each)
    PE  : 4 d-sum matmuls + 1 negI(S16h window) matmul -> psum
    ACT : exp half-group [128,1024] -> P8 bf16 (at rows 3 and 7 of 8)
    DVE : rider ts: dump = P*G, accum_out -> raw32[:, j]   (127ns)
    Pool: cross[k, j+1..j+255] += dump[k, 0:255]           (213ns)
  Riders lag one half-group behind the front so the DVE queue never
  waits on an exp; crosses follow one row behind riders.

  dist psum layout: partition 32c+m holds k=25c+m (m<25); host
  reassembles own rows from raw32 and scatters cross columns t=1..318
  to rows (64c+t) % 512.
"""

import sys
import numpy as np

for _p in ("/opt/trn_rl_repo",):
    if _p not in sys.path:
        sys.path.insert(0, _p)

B = 512
F = 1024
K = 100
D = 5
KD = K * D  # 500
NCORES = 8
JPC = B // NCORES  # 64 output rows per core
NCHUNK = 4  # kd chunks of 125
CHUNK = KD // NCHUNK  # 125
KPC = K // NCHUNK  # 25 k's per chunk
FD = 256  # per-row window: delta = 1..256
W = JPC + FD  # 320 columns of x needed per core
GROUP = 8  # rows per psum wide tile
HALF = GROUP // 2  # rows per exp batch
LAG = 2 * HALF  # rider emission lag (rows)

_NC_CACHE = {}


def build_nc():
    import contextlib

    import concourse.bass as bass
    import concourse.bacc as bacc
    import concourse.mybir as mybir
    from concourse.tile import TileContext

    nc = bacc.Bacc(None, target_bir_lowering=False, debug=True)

    inT = nc.declare_dram_parameter("inT", [F, W], mybir.dt.float16, isOutput=False)
    Tm = nc.declare_dram_parameter("Tm", [F, KD], mybir.dt.float16, isOutput=False)
    onesd = nc.declare_dram_parameter(
        "onesd", [CHUNK, 32], mybir.dt.float16, isOutput=False
    )
    negI = nc.declare_dram_parameter("negI", [128, 128], mybir.dt.float16, isOutput=False)
    raw_out = nc.declare_dram_parameter(
        "raw", [128, JPC], mybir.dt.float32, isOutput=True
    )
    cross_out = nc.declare_dram_parameter(
        "cross", [128, W], mybir.dt.float32, isOutput=True
    )

    with TileContext(nc) as tc:
        with tc.tile_pool(name="persist", bufs=1) as pp:
            T_sb = pp.tile([128, 8 * KD], mybir.dt.float16, name="T_sb")
            inT_sb = pp.tile([128, 8 * W], mybir.dt.float16, name="inT_sb")
            ones_sb = pp.tile([CHUNK, 32], mybir.dt.float16, name="ones_sb")
            negI_sb = pp.tile([128, 128], mybir.dt.float16, name="negI_sb")
            xT_sb = pp.tile([128, NCHUNK * W], mybir.dt.float16, name="xT_sb")
            # f32 upcasts of xT columns 0..JPC (tensor_scalar per-partition
            # scalars must be f32)
            xTj_sb = pp.tile([128, NCHUNK * JPC], mybir.dt.float32, name="xTj_sb")
            S16h_sb = pp.tile([128, W], mybir.dt.float16, name="S16h_sb")
            G_sb = pp.tile([128, JPC], mybir.dt.float32, name="G_sb")
            cross_sb = pp.tile([128, W], mybir.dt.float32, name="cross_sb")
            raw_sb = pp.tile([128, JPC], mybir.dt.float32, name="raw_sb")
            NP8 = 4
            p8_bufs = [
                pp.tile([128, HALF * FD], mybir.dt.bfloat16, name=f"p8_{i}")
                for i in range(NP8)
            ]
            NAB = 16
            ab_bufs = [
                pp.tile([CHUNK, FD], mybir.dt.float16, name=f"ab{i}")
                for i in range(NAB)
            ]
            NDP = 4
            dump_bufs = [
                pp.tile([128, FD], mybir.dt.float16, name=f"dp{i}")
                for i in range(NDP)
            ]

            # warm the ACT exp table while DMAs run (table load ~1.3us)
            warm_sb = pp.tile([1, 1], mybir.dt.float32, name="warm_sb")
            nc.vector.memset(warm_sb[:, :], 0.0)
            nc.scalar.activation(
                warm_sb[:, :], warm_sb[:, :], mybir.ActivationFunctionType.Exp
            )
            nc.vector.memset(cross_sb[:, :], 0.0)

            # --- load inputs: T quarters on the SP queue, inT quarters on
            # the ACT queue so descriptor generation overlaps ---
            for h in range(4):
                nc.sync.dma_start(
                    out=T_sb[:, h * 2 * KD : (h + 1) * 2 * KD],
                    in_=Tm[h * 256 : (h + 1) * 256, :].rearrange(
                        "(t p) c -> p t c", t=2
                    ),
                )
                nc.scalar.dma_start(
                    out=inT_sb[:, h * 2 * W : (h + 1) * 2 * W],
                    in_=inT[h * 256 : (h + 1) * 256, :].rearrange(
                        "(t p) c -> p t c", t=2
                    ),
                )
            nc.sync.dma_start(out=ones_sb[:, :], in_=onesd[:, :])
            nc.sync.dma_start(out=negI_sb[:, :], in_=negI[:, :])

            psum_es = contextlib.ExitStack()
            psum = psum_es.enter_context(
                tc.tile_pool(name="psum", bufs=1, space="PSUM")
            )
            # Two 4-bank-wide dist tiles (all 8 psum banks). The projection
            # aliases its per-chunk accumulators into wide[0] (each chunk in
            # its own bank); S goes into wide[1] before the row loop claims
            # it.
            wide = [
                psum.tile([128, GROUP * FD], mybir.dt.float32, name=f"wide{i}")
                for i in range(2)
            ]
            xt_ps = [wide[0][0:CHUNK, c * 512 : c * 512 + W] for c in range(NCHUNK)]
            S_ps = wide[1][:, 0:W]
            # projection: t-outer for tiles 0..5 (runnable as DMA quarters
            # land), then per-chunk tails so each chunk's psum->sbuf copy
            # starts while the next chunk's tail matmuls run
            for t in range(6):
                for c in range(NCHUNK):
                    nc.tensor.matmul(
                        xt_ps[c],
                        T_sb[:, t * KD + c * CHUNK : t * KD + (c + 1) * CHUNK],
                        inT_sb[:, t * W : (t + 1) * W],
                        start=(t == 0),
                        stop=False,
                        skip_group_check=True,
                    )
            for c in range(NCHUNK):
                for t in (6, 7):
                    nc.tensor.matmul(
                        xt_ps[c],
                        T_sb[:, t * KD + c * CHUNK : t * KD + (c + 1) * CHUNK],
                        inT_sb[:, t * W : (t + 1) * W],
                        start=False,
                        stop=(t == 7),
                        skip_group_check=True,
                    )
                # psum->sbuf copies split between DVE and ACT
                eng = nc.vector.tensor_copy if c % 2 == 0 else nc.scalar.copy
                eng(xT_sb[0:CHUNK, c * W : (c + 1) * W], xt_ps[c])
                if c % 2 == 0:
                    nc.vector.tensor_copy(
                        xTj_sb[0:CHUNK, c * JPC : (c + 1) * JPC],
                        xT_sb[0:CHUNK, c * W : c * W + JPC],
                    )
                else:
                    nc.scalar.copy(
                        xTj_sb[0:CHUNK, c * JPC : (c + 1) * JPC],
                        xT_sb[0:CHUNK, c * W : c * W + JPC],
                    )
                # S[25c+m at partition 32c+m, i] = sum_d x[kd, i]
                nc.tensor.matmul(
                    S_ps[32 * c : 32 * c + 32, :],
                    ones_sb[:, :],
                    xT_sb[0:CHUNK, c * W : (c + 1) * W],
                    start=True,
                    stop=True,
                    tile_position=(0, 32 * c),
                )
            # S16h = S/2 (negI matmul's moving operand: psum gets -S_b/2 so
            # exp(-2*psum) carries exp(+S_b)); G = exp(-S) f32 row scalars
            nc.scalar.mul(S16h_sb[:, :], S_ps[:, :], 0.5)
            nc.scalar.activation(
                G_sb[:, :], S_ps[:, 0:JPC], mybir.ActivationFunctionType.Exp,
                bias=0.0, scale=-1.0,
            )

            # --- main loop over output rows ---
            def emit_front(j):
                g = j // GROUP
                jj = j % GROUP
                dist = wide[g % 2]
                for c in range(NCHUNK):
                    ab = ab_bufs[(j * NCHUNK + c) % NAB]
                    # Pool takes chunk 0 (ready earliest); DVE the rest
                    eng = nc.gpsimd if c == 0 else nc.vector
                    eng.tensor_scalar(
                        ab[:, :],
                        xT_sb[0:CHUNK, c * W + j + 1 : c * W + j + 1 + FD],
                        xTj_sb[0:CHUNK, c * JPC + j : c * JPC + j + 1],
                        0.0,
                        mybir.AluOpType.subtract,
                        mybir.AluOpType.max,
                    )
                    nc.tensor.matmul(
                        dist[32 * c : 32 * c + 32, jj * FD : (jj + 1) * FD],
                        ones_sb[:, :],
                        ab[:, :],
                        start=True,
                        stop=False,
                        tile_position=(0, 32 * c),
                        skip_group_check=True,
                    )
                # psum -= S16h[., j+1 : j+257]   (closes all 4 groups)
                nc.tensor.matmul(
                    dist[:, jj * FD : (jj + 1) * FD],
                    negI_sb[:, :],
                    S16h_sb[:, j + 1 : j + 1 + FD],
                    start=False,
                    stop=True,
                    skip_group_check=True,
                )
                if jj % HALF == HALF - 1:
                    # batched P = exp(-2*psum) over a 4-row half-group
                    h = jj // HALF
                    hg = 2 * g + h
                    nc.scalar.activation(
                        p8_bufs[hg % NP8][:, :],
                        dist[:, h * HALF * FD : (h + 1) * HALF * FD],
                        mybir.ActivationFunctionType.Exp,
                        bias=0.0,
                        scale=-2.0,
                    )

            def emit_rider(j):
                hg = j // HALF
                jj = j % HALF
                p8 = p8_bufs[hg % NP8]
                # dump = P * exp(-S_a);  accum_out = row sum (free on 4x)
                nc.vector.tensor_scalar(
                    dump_bufs[j % NDP][:, :],
                    p8[:, jj * FD : (jj + 1) * FD],
                    G_sb[:, j : j + 1],
                    0.0,
                    mybir.AluOpType.mult,
                    mybir.AluOpType.add,
                    accum_out=raw_sb[:, j : j + 1],
                )

            def emit_cross(j):
                # cross[k, j+delta] += dump[k, delta-1] for delta = 1..255
                # (delta=256 belongs to the partner row's own sum)
                nc.gpsimd.tensor_tensor(
                    cross_sb[:, j + 1 : j + FD],
                    cross_sb[:, j + 1 : j + FD],
                    dump_bufs[j % NDP][:, 0 : FD - 1],
                    mybir.AluOpType.add,
                )

            for j in range(JPC + LAG + 1):
                if j < JPC:
                    emit_front(j)
                if LAG <= j < JPC + LAG:
                    emit_rider(j - LAG)
                if j > LAG:
                    emit_cross(j - LAG - 1)

            psum_es.close()
            nc.scalar.dma_start(out=raw_out[:, :], in_=raw_sb[:, :])
            nc.sync.dma_start(out=cross_out[:, :], in_=cross_sb[:, :])

    nc.finalize()
    return nc


def _aux_consts():
    ob = np.zeros([CHUNK, 32], dtype=np.float16)
    for m in range(KPC):
        ob[5 * m : 5 * m + 5, m] = 1.0
    negI = (-np.eye(128)).astype(np.float16)
    return ob, negI


def make_in_maps(inputs, T):
    f16 = np.float16
    Tm = np.asarray(T, dtype=np.float32).astype(f16)
    ob, negI = _aux_consts()
    in_maps = []
    for c in range(NCORES):
        rolled = np.roll(np.asarray(inputs, dtype=np.float32), -JPC * c, axis=0)
        inTc = np.ascontiguousarray(rolled[0:W].T).astype(f16)
        in_maps.append(
            {
                "inT": inTc,
                "Tm": Tm,
                "onesd": ob,
                "negI": negI,
            }
        )
    return in_maps


def assemble_output(results):
    out = np.zeros([B, K], dtype=np.float32)
    for c in range(NCORES):
        rawc = np.asarray(results[c]["raw"], dtype=np.float32)  # [128, JPC]
        cross = np.asarray(results[c]["cross"], dtype=np.float32)  # [128, W]
        for cc in range(NCHUNK):
            ksl = slice(32 * cc, 32 * cc + KPC)
            kg = slice(KPC * cc, KPC * (cc + 1))
            # own rows: global rows 64c..64c+63 (+1.0 self term)
            out[JPC * c : JPC * (c + 1), kg] += rawc[ksl, :].T + 1.0
            # cross rows: global rows (64c + t) % 512 for t = 1..W-1
            rows = (JPC * c + np.arange(1, W)) % B
            np.add.at(
                out,
                (rows[:, None], np.arange(KPC * cc, KPC * (cc + 1))[None, :]),
                cross[ksl, 1:W].T,
            )
    return out


def kernel(inputs, T):
    from concourse.bass_utils import run_bass_kernel_spmd

    if "nc" not in _NC_CACHE:
        _NC_CACHE["nc"] = build_nc()
    nc = _NC_CACHE["nc"]
    in_maps = make_in_maps(inputs, T)
    res = run_bass_kernel_spmd(nc, in_maps, list(range(NCORES)))
    return assemble_output(res.results)


if __name__ == "__main__":
    sys.path.insert(0, "/root/problem")
    from reference import setup_inputs, reference

    inputs = setup_inputs()
    expected = np.asarray(reference(**inputs))
    actual = kernel(**{k: np.asarray(v) for k, v in inputs.items()})
    err = np.abs(actual - expected)
    rel = np.linalg.norm(actual - expected) / np.linalg.norm(expected)
    print(f"max abs err: {err.max():.3e}")
    print(f"Relative error: {rel:.3e}")


# revision 16
# speedup vs baseline: 1.1361x; 1.1361x over previous
"""
MinibatchDiscrimination kernel for 8x TRN2 NeuronCores (Bass/Tile).

Math:  x = inputs @ T  -> [B, K, D] with B=512, K=100, D=5
       out[a,k] = sum_b exp(-sum_d |x[a,k,d]-x[b,k,d]|)

v4 strategy (v2 = 49.8us, v3 experiments showed per-row ACT exp overhead
and batched-multiply chains were the limiters):

  Pair coverage (as v2): core c owns global rows a = 64c+j (j=0..63) and
  window delta = 1..256 (partners b = a+delta mod 512). Deltas 1..255
  cover each unordered pair once; delta=256 pairs appear from both
  endpoints, and each endpoint keeps its own copy in its row sum while
  the cross path scatters only delta=1..255, so no correction columns
  are needed. The self term exp(0)=1 is added on the host.

  Identity: |u-v| = 2*relu(u-v) - u + v  =>  dist = 2R' - S_b + S_a with
  R' = sum_d relu(x_b - x_a), S = sum_d x. PSUM accumulates
  (R' - S_b/2) per row (4 ones-matmuls + one -I matmul with the halved
  S16 window), so one BIAS-FREE activation computes
      P = exp(-2*psum) = exp(S_b - 2R') = exp(S_a - dist)
  batched over FOUR rows at a time ([128,1024] spanning 2 psum banks,
  260ns/row vs v2's 585ns/row exp+accum). P can reach exp(+17) so P8 is
  stored in BF16 (8-bit exponent - no overflow; 0.4% mantissa error is
  well inside tolerance). A single DVE tensor_scalar rider per row then
  applies the row factor and the row sum in one 4x-mode op:
      dump = P * exp(-S_a)   (G scalar, f32),  accum_out = sum(dump).

  Per row j:
    Pool: relu chunk 0 (ready earliest from the projection), 213ns
    DVE : relu chunks 1,2,3 (fp16 4x mode, 127ns each)
    PE  : 4 d-sum matmuls + 1 negI(S16h window) matmul -> psum
    ACT : exp half-group [128,1024] -> P8 bf16 (at rows 3 and 7 of 8)
    DVE : rider ts: dump = P*G, accum_out -> raw32[:, j]   (127ns)
    Pool: cross[k, j+1..j+255] += dump[k, 0:255]           (213ns)
  Riders lag one half-group behind the front so the DVE queue never
  waits on an exp; crosses follow one row behind riders.

  dist psum layout: partition 32c+m holds k=25c+m (m<25); host
  reassembles own rows from raw32 and scatters cross columns t=1..318
  to rows (64c+t) % 512.
"""

import sys
import numpy as np

for _p in ("/opt/trn_rl_repo",):
    if _p not in sys.path:
        sys.path.insert(0, _p)

B = 512
F = 1024
K = 100
D = 5
KD = K * D  # 500
NCORES = 8
JPC = B // NCORES  # 64 output rows per core
NCHUNK = 4  # kd chunks of 125
CHUNK = KD // NCHUNK  # 125
KPC = K // NCHUNK  # 25 k's per chunk
FD = 256  # per-row window: delta = 1..256
W = JPC + FD  # 320 columns of x needed per core
GROUP = 8  # rows per psum wide tile
HALF = GROUP // 2  # rows per exp batch
LAG = 2 * HALF  # rider emission lag (rows)

_NC_CACHE = {}


def build_nc():
    import contextlib

    import concourse.bass as bass
    import concourse.bacc as bacc
    import concourse.mybir as mybir
    from concourse.tile import TileContext

    nc = bacc.Bacc(None, target_bir_lowering=False, debug=True)

    inT = nc.declare_dram_parameter("inT", [F, W], mybir.dt.float16, isOutput=False)
    Tm = nc.declare_dram_parameter("Tm", [F, KD], mybir.dt.float16, isOutput=False)
    onesd = nc.declare_dram_parameter(
        "onesd", [CHUNK, 32], mybir.dt.float16, isOutput=False
    )
    negI = nc.declare_dram_parameter("negI", [128, 128], mybir.dt.float16, isOutput=False)
    raw_out = nc.declare_dram_parameter(
        "raw", [128, JPC], mybir.dt.float32, isOutput=True
    )
    cross_out = nc.declare_dram_parameter(
        "cross", [128, W], mybir.dt.float32, isOutput=True
    )

    with TileContext(nc) as tc:
        with tc.tile_pool(name="persist", bufs=1) as pp:
            T_sb = pp.tile([128, 8 * KD], mybir.dt.float16, name="T_sb")
            inT_sb = pp.tile([128, 8 * W], mybir.dt.float16, name="inT_sb")
            ones_sb = pp.tile([CHUNK, 32], mybir.dt.float16, name="ones_sb")
            negI_sb = pp.tile([128, 128], mybir.dt.float16, name="negI_sb")
            xT_sb = pp.tile([128, NCHUNK * W], mybir.dt.float16, name="xT_sb")
            # f32 upcasts of xT columns 0..JPC (tensor_scalar per-partition
            # scalars must be f32)
            xTj_sb = pp.tile([128, NCHUNK * JPC], mybir.dt.float32, name="xTj_sb")
            S16h_sb = pp.tile([128, W], mybir.dt.float16, name="S16h_sb")
            G_sb = pp.tile([128, JPC], mybir.dt.float32, name="G_sb")
            cross_sb = pp.tile([128, W], mybir.dt.float32, name="cross_sb")
            raw_sb = pp.tile([128, JPC], mybir.dt.float32, name="raw_sb")
            NP8 = 6
            p8_bufs = [
                pp.tile([128, HALF * FD], mybir.dt.bfloat16, name=f"p8_{i}")
                for i in range(NP8)
            ]
            NAB = 32
            ab_bufs = [
                pp.tile([CHUNK, FD], mybir.dt.float16, name=f"ab{i}")
                for i in range(NAB)
            ]
            NDP = 8
            dump_bufs = [
                pp.tile([128, FD], mybir.dt.float16, name=f"dp{i}")
                for i in range(NDP)
            ]

            # warm the ACT exp table while DMAs run (table load ~1.3us)
            warm_sb = pp.tile([1, 1], mybir.dt.float32, name="warm_sb")
            nc.vector.memset(warm_sb[:, :], 0.0)
            nc.scalar.activation(
                warm_sb[:, :], warm_sb[:, :], mybir.ActivationFunctionType.Exp
            )
            nc.vector.memset(cross_sb[:, :], 0.0)

            # --- load inputs: T quarters on the SP queue, inT quarters on
            # the ACT queue so descriptor generation overlaps ---
            for h in range(4):
                nc.sync.dma_start(
                    out=T_sb[:, h * 2 * KD : (h + 1) * 2 * KD],
                    in_=Tm[h * 256 : (h + 1) * 256, :].rearrange(
                        "(t p) c -> p t c", t=2
                    ),
                )
                nc.scalar.dma_start(
                    out=inT_sb[:, h * 2 * W : (h + 1) * 2 * W],
                    in_=inT[h * 256 : (h + 1) * 256, :].rearrange(
                        "(t p) c -> p t c", t=2
                    ),
                )
            nc.sync.dma_start(out=ones_sb[:, :], in_=onesd[:, :])
            nc.sync.dma_start(out=negI_sb[:, :], in_=negI[:, :])

            psum_es = contextlib.ExitStack()
            psum = psum_es.enter_context(
                tc.tile_pool(name="psum", bufs=1, space="PSUM")
            )
            # Two 4-bank-wide dist tiles (all 8 psum banks). The projection
            # aliases its per-chunk accumulators into wide[0] (each chunk in
            # its own bank); S goes into wide[1] before the row loop claims
            # it.
            wide = [
                psum.tile([128, GROUP * FD], mybir.dt.float32, name=f"wide{i}")
                for i in range(2)
            ]
            xt_ps = [wide[0][0:CHUNK, c * 512 : c * 512 + W] for c in range(NCHUNK)]
            S_ps = wide[1][:, 0:W]
            # projection: t-outer for tiles 0..5 (runnable as DMA quarters
            # land), then per-chunk tails so each chunk's psum->sbuf copy
            # starts while the next chunk's tail matmuls run
            for t in range(6):
                for c in range(NCHUNK):
                    nc.tensor.matmul(
                        xt_ps[c],
                        T_sb[:, t * KD + c * CHUNK : t * KD + (c + 1) * CHUNK],
                        inT_sb[:, t * W : (t + 1) * W],
                        start=(t == 0),
                        stop=False,
                        skip_group_check=True,
                    )
            for c in range(NCHUNK):
                for t in (6, 7):
                    nc.tensor.matmul(
                        xt_ps[c],
                        T_sb[:, t * KD + c * CHUNK : t * KD + (c + 1) * CHUNK],
                        inT_sb[:, t * W : (t + 1) * W],
                        start=False,
                        stop=(t == 7),
                        skip_group_check=True,
                    )
                # psum->sbuf copies split between DVE and ACT
                eng = nc.vector.tensor_copy if c % 2 == 0 else nc.scalar.copy
                eng(xT_sb[0:CHUNK, c * W : (c + 1) * W], xt_ps[c])
                if c % 2 == 0:
                    nc.vector.tensor_copy(
                        xTj_sb[0:CHUNK, c * JPC : (c + 1) * JPC],
                        xT_sb[0:CHUNK, c * W : c * W + JPC],
                    )
                else:
                    nc.scalar.copy(
                        xTj_sb[0:CHUNK, c * JPC : (c + 1) * JPC],
                        xT_sb[0:CHUNK, c * W : c * W + JPC],
                    )
                # S[25c+m at partition 32c+m, i] = sum_d x[kd, i]
                nc.tensor.matmul(
                    S_ps[32 * c : 32 * c + 32, :],
                    ones_sb[:, :],
                    xT_sb[0:CHUNK, c * W : (c + 1) * W],
                    start=True,
                    stop=True,
                    tile_position=(0, 32 * c),
                )
            # S16h = S/2 (negI matmul's moving operand: psum gets -S_b/2 so
            # exp(-2*psum) carries exp(+S_b)); G = exp(-S) f32 row scalars
            nc.scalar.mul(S16h_sb[:, :], S_ps[:, :], 0.5)
            nc.scalar.activation(
                G_sb[:, :], S_ps[:, 0:JPC], mybir.ActivationFunctionType.Exp,
                bias=0.0, scale=-1.0,
            )

            # --- main loop over output rows ---
            def emit_front(j):
                g = j // GROUP
                jj = j % GROUP
                dist = wide[g % 2]
                for c in range(NCHUNK):
                    ab = ab_bufs[(j * NCHUNK + c) % NAB]
                    # Pool takes chunk 0 (ready earliest); DVE the rest
                    eng = nc.gpsimd if c == 0 else nc.vector
                    eng.tensor_scalar(
                        ab[:, :],
                        xT_sb[0:CHUNK, c * W + j + 1 : c * W + j + 1 + FD],
                        xTj_sb[0:CHUNK, c * JPC + j : c * JPC + j + 1],
                        0.0,
                        mybir.AluOpType.subtract,
                        mybir.AluOpType.max,
                    )
                    nc.tensor.matmul(
                        dist[32 * c : 32 * c + 32, jj * FD : (jj + 1) * FD],
                        ones_sb[:, :],
                        ab[:, :],
                        start=True,
                        stop=False,
                        tile_position=(0, 32 * c),
                        skip_group_check=True,
                    )
                # psum -= S16h[., j+1 : j+257]   (closes all 4 groups)
                nc.tensor.matmul(
                    dist[:, jj * FD : (jj + 1) * FD],
                    negI_sb[:, :],
                    S16h_sb[:, j + 1 : j + 1 + FD],
                    start=False,
                    stop=True,
                    skip_group_check=True,
                )
                if jj % HALF == HALF - 1:
                    # batched P = exp(-2*psum) over a 4-row half-group
                    h = jj // HALF
                    hg = 2 * g + h
                    nc.scalar.activation(
                        p8_bufs[hg % NP8][:, :],
                        dist[:, h * HALF * FD : (h + 1) * HALF * FD],
                        mybir.ActivationFunctionType.Exp,
                        bias=0.0,
                        scale=-2.0,
                    )

            def emit_rider(j):
                hg = j // HALF
                jj = j % HALF
                p8 = p8_bufs[hg % NP8]
                # dump = P * exp(-S_a);  accum_out = row sum (free on 4x)
                nc.vector.tensor_scalar(
                    dump_bufs[j % NDP][:, :],
                    p8[:, jj * FD : (jj + 1) * FD],
                    G_sb[:, j : j + 1],
                    0.0,
                    mybir.AluOpType.mult,
                    mybir.AluOpType.add,
                    accum_out=raw_sb[:, j : j + 1],
                )

            def emit_cross(j):
                # cross[k, j+delta] += dump[k, delta-1] for delta = 1..255
                # (delta=256 belongs to the partner row's own sum)
                nc.gpsimd.tensor_tensor(
                    cross_sb[:, j + 1 : j + FD],
                    cross_sb[:, j + 1 : j + FD],
                    dump_bufs[j % NDP][:, 0 : FD - 1],
                    mybir.AluOpType.add,
                )

            for j in range(JPC + LAG + 1):
                if j < JPC:
                    emit_front(j)
                if LAG <= j < JPC + LAG:
                    emit_rider(j - LAG)
                if j > LAG:
                    emit_cross(j - LAG - 1)

            psum_es.close()
            nc.scalar.dma_start(out=raw_out[:, :], in_=raw_sb[:, :])
            nc.sync.dma_start(out=cross_out[:, :], in_=cross_sb[:, :])

    nc.finalize()
    return nc


def _aux_consts():
    ob = np.zeros([CHUNK, 32], dtype=np.float16)
    for m in range(KPC):
        ob[5 * m : 5 * m + 5, m] = 1.0
    negI = (-np.eye(128)).astype(np.float16)
    return ob, negI


def make_in_maps(inputs, T):
    f16 = np.float16
    Tm = np.asarray(T, dtype=np.float32).astype(f16)
    ob, negI = _aux_consts()
    in_maps = []
    for c in range(NCORES):
        rolled = np.roll(np.asarray(inputs, dtype=np.float32), -JPC * c, axis=0)
        inTc = np.ascontiguousarray(rolled[0:W].T).astype(f16)
        in_maps.append(
            {
                "inT": inTc,
                "Tm": Tm,
                "onesd": ob,
                "negI": negI,
            }
        )
    return in_maps


def assemble_output(results):
    out = np.zeros([B, K], dtype=np.float32)
    for c in range(NCORES):
        rawc = np.asarray(results[c]["raw"], dtype=np.float32)  # [128, JPC]
        cross = np.asarray(results[c]["cross"], dtype=np.float32)  # [128, W]
        for cc in range(NCHUNK):
            ksl = slice(32 * cc, 32 * cc + KPC)
            kg = slice(KPC * cc, KPC * (cc + 1))
            # own rows: global rows 64c..64c+63 (+1.0 self term)
            out[JPC * c : JPC * (c + 1), kg] += rawc[ksl, :].T + 1.0
            # cross rows: global rows (64c + t) % 512 for t = 1..W-1
            rows = (JPC * c + np.arange(1, W)) % B
            np.add.at(
                out,
                (rows[:, None], np.arange(KPC * cc, KPC * (cc + 1))[None, :]),
                cross[ksl, 1:W].T,
            )
    return out


def kernel(inputs, T):
    from concourse.bass_utils import run_bass_kernel_spmd

    if "nc" not in _NC_CACHE:
        _NC_CACHE["nc"] = build_nc()
    nc = _NC_CACHE["nc"]
    in_maps = make_in_maps(inputs, T)
    res = run_bass_kernel_spmd(nc, in_maps, list(range(NCORES)))
    return assemble_output(res.results)


if __name__ == "__main__":
    sys.path.insert(0, "/root/problem")
    from reference import setup_inputs, reference

    inputs = setup_inputs()
    expected = np.asarray(reference(**inputs))
    actual = kernel(**{k: np.asarray(v) for k, v in inputs.items()})
    err = np.abs(actual - expected)
    rel = np.linalg.norm(actual - expected) / np.linalg.norm(expected)
    print(f"max abs err: {err.max():.3e}")
    print(f"Relative error: {rel:.3e}")


# revision 17
# speedup vs baseline: 1.2453x; 1.0961x over previous
"""
MinibatchDiscrimination kernel for 8x TRN2 NeuronCores (Bass/Tile).

Math:  x = inputs @ T  -> [B, K, D] with B=512, K=100, D=5
       out[a,k] = sum_b exp(-sum_d |x[a,k,d]-x[b,k,d]|)

v4 strategy (v2 = 49.8us, v3 experiments showed per-row ACT exp overhead
and batched-multiply chains were the limiters):

  Pair coverage (as v2): core c owns global rows a = 64c+j (j=0..63) and
  window delta = 1..256 (partners b = a+delta mod 512). Deltas 1..255
  cover each unordered pair once; delta=256 pairs appear from both
  endpoints, and each endpoint keeps its own copy in its row sum while
  the cross path scatters only delta=1..255, so no correction columns
  are needed. The self term exp(0)=1 is added on the host.

  Identity: |u-v| = 2*relu(u-v) - u + v  =>  dist = 2R' - S_b + S_a with
  R' = sum_d relu(x_b - x_a), S = sum_d x. PSUM accumulates
  (R' - S_b/2) per row (4 ones-matmuls + one -I matmul with the halved
  S16 window), so one BIAS-FREE activation computes
      P = exp(-2*psum) = exp(S_b - 2R') = exp(S_a - dist)
  batched over FOUR rows at a time ([128,1024] spanning 2 psum banks,
  260ns/row vs v2's 585ns/row exp+accum). P can reach exp(+17) so P8 is
  stored in BF16 (8-bit exponent - no overflow; 0.4% mantissa error is
  well inside tolerance). A single DVE tensor_scalar rider per row then
  applies the row factor and the row sum in one 4x-mode op:
      dump = P * exp(-S_a)   (G scalar, f32),  accum_out = sum(dump).

  Per row j:
    Pool: relu chunk 0 (ready earliest from the projection), 213ns
    DVE : relu chunks 1,2,3 (fp16 4x mode, 127ns each)
    PE  : 4 d-sum matmuls + 1 negI(S16h window) matmul -> psum
    ACT : exp half-group [128,1024] -> P8 bf16 (at rows 3 and 7 of 8)
    DVE : rider ts: dump = P*G, accum_out -> raw32[:, j]   (127ns)
    Pool: cross[k, j+1..j+255] += dump[k, 0:255]           (213ns)
  Riders lag one half-group behind the front so the DVE queue never
  waits on an exp; crosses follow one row behind riders.

  dist psum layout: partition 32c+m holds k=25c+m (m<25); host
  reassembles own rows from raw32 and scatters cross columns t=1..318
  to rows (64c+t) % 512.
"""

import sys
import numpy as np

for _p in ("/opt/trn_rl_repo",):
    if _p not in sys.path:
        sys.path.insert(0, _p)

B = 512
F = 1024
K = 100
D = 5
KD = K * D  # 500
NCORES = 8
JPC = B // NCORES  # 64 output rows per core
NCHUNK = 4  # kd chunks of 125
CHUNK = KD // NCHUNK  # 125
KPC = K // NCHUNK  # 25 k's per chunk
FD = 256  # per-row window: delta = 1..256
W = JPC + FD  # 320 columns of x needed per core
GROUP = 8  # rows per psum wide tile
HALF = GROUP // 2  # rows per exp batch
LAG = 2 * HALF  # rider emission lag (rows)

_NC_CACHE = {}


def build_nc():
    import contextlib

    import concourse.bass as bass
    import concourse.bacc as bacc
    import concourse.mybir as mybir
    from concourse.tile import TileContext

    nc = bacc.Bacc(None, target_bir_lowering=False, debug=True)

    inT = nc.declare_dram_parameter("inT", [F, W], mybir.dt.float16, isOutput=False)
    Tm = nc.declare_dram_parameter("Tm", [F, KD], mybir.dt.float16, isOutput=False)
    onesd = nc.declare_dram_parameter(
        "onesd", [CHUNK, 32], mybir.dt.float16, isOutput=False
    )
    negI = nc.declare_dram_parameter("negI", [128, 128], mybir.dt.float16, isOutput=False)
    raw_out = nc.declare_dram_parameter(
        "raw", [128, JPC], mybir.dt.float32, isOutput=True
    )
    cross_out = nc.declare_dram_parameter(
        "cross", [128, W], mybir.dt.float32, isOutput=True
    )

    with TileContext(nc) as tc:
        with tc.tile_pool(name="persist", bufs=1) as pp:
            T_sb = pp.tile([128, 8 * KD], mybir.dt.float16, name="T_sb")
            inT_sb = pp.tile([128, 8 * W], mybir.dt.float16, name="inT_sb")
            ones_sb = pp.tile([CHUNK, 32], mybir.dt.float16, name="ones_sb")
            negI_sb = pp.tile([128, 128], mybir.dt.float16, name="negI_sb")
            xT_sb = pp.tile([128, NCHUNK * W], mybir.dt.float16, name="xT_sb")
            # f32 upcasts of xT columns 0..JPC (tensor_scalar per-partition
            # scalars must be f32)
            xTj_sb = pp.tile([128, NCHUNK * JPC], mybir.dt.float32, name="xTj_sb")
            S16h_sb = pp.tile([128, W], mybir.dt.float16, name="S16h_sb")
            G_sb = pp.tile([128, JPC], mybir.dt.float32, name="G_sb")
            cross_sb = pp.tile([128, W], mybir.dt.float32, name="cross_sb")
            raw_sb = pp.tile([128, JPC], mybir.dt.float32, name="raw_sb")
            NP8 = 6
            p8_bufs = [
                pp.tile([128, HALF * FD], mybir.dt.bfloat16, name=f"p8_{i}")
                for i in range(NP8)
            ]
            NAB = 48
            ab_bufs = [
                pp.tile([CHUNK, FD], mybir.dt.float16, name=f"ab{i}")
                for i in range(NAB)
            ]
            NDP = 8
            dump_bufs = [
                pp.tile([128, FD], mybir.dt.float16, name=f"dp{i}")
                for i in range(NDP)
            ]

            # warm the ACT exp table while DMAs run (table load ~1.3us)
            warm_sb = pp.tile([1, 1], mybir.dt.float32, name="warm_sb")
            nc.vector.memset(warm_sb[:, :], 0.0)
            nc.scalar.activation(
                warm_sb[:, :], warm_sb[:, :], mybir.ActivationFunctionType.Exp
            )
            nc.vector.memset(cross_sb[:, :], 0.0)

            # --- load inputs: T quarters on the SP queue, inT quarters on
            # the ACT queue so descriptor generation overlaps ---
            for h in range(4):
                nc.sync.dma_start(
                    out=T_sb[:, h * 2 * KD : (h + 1) * 2 * KD],
                    in_=Tm[h * 256 : (h + 1) * 256, :].rearrange(
                        "(t p) c -> p t c", t=2
                    ),
                )
                nc.scalar.dma_start(
                    out=inT_sb[:, h * 2 * W : (h + 1) * 2 * W],
                    in_=inT[h * 256 : (h + 1) * 256, :].rearrange(
                        "(t p) c -> p t c", t=2
                    ),
                )
            nc.sync.dma_start(out=ones_sb[:, :], in_=onesd[:, :])
            nc.sync.dma_start(out=negI_sb[:, :], in_=negI[:, :])

            psum_es = contextlib.ExitStack()
            psum = psum_es.enter_context(
                tc.tile_pool(name="psum", bufs=1, space="PSUM")
            )
            # Four 2-bank-wide dist tiles (all 8 psum banks), one per
            # half-group so WAR tracking decouples each exp from the next
            # halves' matmuls. The projection aliases chunk c's accumulator
            # into wide[c]; S goes into wide[3]'s second bank.
            wide = [
                psum.tile([128, HALF * FD], mybir.dt.float32, name=f"wide{i}")
                for i in range(4)
            ]
            xt_ps = [wide[c][0:CHUNK, 0:W] for c in range(NCHUNK)]
            S_ps = wide[3][:, 512 : 512 + W]
            # projection: t-outer for tiles 0..5 (runnable as DMA quarters
            # land), then per-chunk tails so each chunk's psum->sbuf copy
            # starts while the next chunk's tail matmuls run
            for t in range(6):
                for c in range(NCHUNK):
                    nc.tensor.matmul(
                        xt_ps[c],
                        T_sb[:, t * KD + c * CHUNK : t * KD + (c + 1) * CHUNK],
                        inT_sb[:, t * W : (t + 1) * W],
                        start=(t == 0),
                        stop=False,
                        skip_group_check=True,
                    )
            for c in range(NCHUNK):
                for t in (6, 7):
                    nc.tensor.matmul(
                        xt_ps[c],
                        T_sb[:, t * KD + c * CHUNK : t * KD + (c + 1) * CHUNK],
                        inT_sb[:, t * W : (t + 1) * W],
                        start=False,
                        stop=(t == 7),
                        skip_group_check=True,
                    )
                # psum->sbuf copies split between DVE and ACT
                eng = nc.vector.tensor_copy if c % 2 == 0 else nc.scalar.copy
                eng(xT_sb[0:CHUNK, c * W : (c + 1) * W], xt_ps[c])
                if c % 2 == 0:
                    nc.vector.tensor_copy(
                        xTj_sb[0:CHUNK, c * JPC : (c + 1) * JPC],
                        xT_sb[0:CHUNK, c * W : c * W + JPC],
                    )
                else:
                    nc.scalar.copy(
                        xTj_sb[0:CHUNK, c * JPC : (c + 1) * JPC],
                        xT_sb[0:CHUNK, c * W : c * W + JPC],
                    )
                # S[25c+m at partition 32c+m, i] = sum_d x[kd, i]
                nc.tensor.matmul(
                    S_ps[32 * c : 32 * c + 32, :],
                    ones_sb[:, :],
                    xT_sb[0:CHUNK, c * W : (c + 1) * W],
                    start=True,
                    stop=True,
                    tile_position=(0, 32 * c),
                )
            # S16h = S/2 (negI matmul's moving operand: psum gets -S_b/2 so
            # exp(-2*psum) carries exp(+S_b)); G = exp(-S) f32 row scalars
            nc.scalar.mul(S16h_sb[:, :], S_ps[:, :], 0.5)
            nc.scalar.activation(
                G_sb[:, :], S_ps[:, 0:JPC], mybir.ActivationFunctionType.Exp,
                bias=0.0, scale=-1.0,
            )

            # --- main loop over output rows ---
            def emit_front(j):
                hh = j // HALF
                jj = j % HALF
                dist = wide[hh % 4]
                for c in range(NCHUNK):
                    ab = ab_bufs[(j * NCHUNK + c) % NAB]
                    # Pool takes chunk 0 (ready earliest); DVE the rest
                    eng = nc.gpsimd if c == 0 else nc.vector
                    eng.tensor_scalar(
                        ab[:, :],
                        xT_sb[0:CHUNK, c * W + j + 1 : c * W + j + 1 + FD],
                        xTj_sb[0:CHUNK, c * JPC + j : c * JPC + j + 1],
                        0.0,
                        mybir.AluOpType.subtract,
                        mybir.AluOpType.max,
                    )
                    nc.tensor.matmul(
                        dist[32 * c : 32 * c + 32, jj * FD : (jj + 1) * FD],
                        ones_sb[:, :],
                        ab[:, :],
                        start=True,
                        stop=False,
                        tile_position=(0, 32 * c),
                        skip_group_check=True,
                    )
                # psum -= S16h[., j+1 : j+257]   (closes all 4 groups)
                nc.tensor.matmul(
                    dist[:, jj * FD : (jj + 1) * FD],
                    negI_sb[:, :],
                    S16h_sb[:, j + 1 : j + 1 + FD],
                    start=False,
                    stop=True,
                    skip_group_check=True,
                )
                if jj == HALF - 1:
                    # batched P = exp(-2*psum) over a 4-row half-group
                    nc.scalar.activation(
                        p8_bufs[hh % NP8][:, :],
                        dist[:, :],
                        mybir.ActivationFunctionType.Exp,
                        bias=0.0,
                        scale=-2.0,
                    )

            def emit_rider(j):
                hg = j // HALF
                jj = j % HALF
                p8 = p8_bufs[hg % NP8]
                # dump = P * exp(-S_a);  accum_out = row sum (free on 4x)
                nc.vector.tensor_scalar(
                    dump_bufs[j % NDP][:, :],
                    p8[:, jj * FD : (jj + 1) * FD],
                    G_sb[:, j : j + 1],
                    0.0,
                    mybir.AluOpType.mult,
                    mybir.AluOpType.add,
                    accum_out=raw_sb[:, j : j + 1],
                )

            def emit_cross(j):
                # cross[k, j+delta] += dump[k, delta-1] for delta = 1..255
                # (delta=256 belongs to the partner row's own sum)
                nc.gpsimd.tensor_tensor(
                    cross_sb[:, j + 1 : j + FD],
                    cross_sb[:, j + 1 : j + FD],
                    dump_bufs[j % NDP][:, 0 : FD - 1],
                    mybir.AluOpType.add,
                )

            for j in range(JPC + LAG + 1):
                if j < JPC:
                    emit_front(j)
                if LAG <= j < JPC + LAG:
                    emit_rider(j - LAG)
                if j > LAG:
                    emit_cross(j - LAG - 1)

            psum_es.close()
            nc.scalar.dma_start(out=raw_out[:, :], in_=raw_sb[:, :])
            nc.sync.dma_start(out=cross_out[:, :], in_=cross_sb[:, :])

    nc.finalize()
    return nc


def _aux_consts():
    ob = np.zeros([CHUNK, 32], dtype=np.float16)
    for m in range(KPC):
        ob[5 * m : 5 * m + 5, m] = 1.0
    negI = (-np.eye(128)).astype(np.float16)
    return ob, negI


def make_in_maps(inputs, T):
    f16 = np.float16
    Tm = np.asarray(T, dtype=np.float32).astype(f16)
    ob, negI = _aux_consts()
    in_maps = []
    for c in range(NCORES):
        rolled = np.roll(np.asarray(inputs, dtype=np.float32), -JPC * c, axis=0)
        inTc = np.ascontiguousarray(rolled[0:W].T).astype(f16)
        in_maps.append(
            {
                "inT": inTc,
                "Tm": Tm,
                "onesd": ob,
                "negI": negI,
            }
        )
    return in_maps


def assemble_output(results):
    out = np.zeros([B, K], dtype=np.float32)
    for c in range(NCORES):
        rawc = np.asarray(results[c]["raw"], dtype=np.float32)  # [128, JPC]
        cross = np.asarray(results[c]["cross"], dtype=np.float32)  # [128, W]
        for cc in range(NCHUNK):
            ksl = slice(32 * cc, 32 * cc + KPC)
            kg = slice(KPC * cc, KPC * (cc + 1))
            # own rows: global rows 64c..64c+63 (+1.0 self term)
            out[JPC * c : JPC * (c + 1), kg] += rawc[ksl, :].T + 1.0
            # cross rows: global rows (64c + t) % 512 for t = 1..W-1
            rows = (JPC * c + np.arange(1, W)) % B
            np.add.at(
                out,
                (rows[:, None], np.arange(KPC * cc, KPC * (cc + 1))[None, :]),
                cross[ksl, 1:W].T,
            )
    return out


def kernel(inputs, T):
    from concourse.bass_utils import run_bass_kernel_spmd

    if "nc" not in _NC_CACHE:
        _NC_CACHE["nc"] = build_nc()
    nc = _NC_CACHE["nc"]
    in_maps = make_in_maps(inputs, T)
    res = run_bass_kernel_spmd(nc, in_maps, list(range(NCORES)))
    return assemble_output(res.results)


if __name__ == "__main__":
    sys.path.insert(0, "/root/problem")
    from reference import setup_inputs, reference

    inputs = setup_inputs()
    expected = np.asarray(reference(**inputs))
    actual = kernel(**{k: np.asarray(v) for k, v in inputs.items()})
    err = np.abs(actual - expected)
    rel = np.linalg.norm(actual - expected) / np.linalg.norm(expected)
    print(f"max abs err: {err.max():.3e}")
    print(f"Relative error: {rel:.3e}")


# revision 18
# speedup vs baseline: 1.2697x; 1.0196x over previous
"""
MinibatchDiscrimination kernel for 8x TRN2 NeuronCores (Bass/Tile).

Math:  x = inputs @ T  -> [B, K, D] with B=512, K=100, D=5
       out[a,k] = sum_b exp(-sum_d |x[a,k,d]-x[b,k,d]|)

v4 strategy (v2 = 49.8us, v3 experiments showed per-row ACT exp overhead
and batched-multiply chains were the limiters):

  Pair coverage (as v2): core c owns global rows a = 64c+j (j=0..63) and
  window delta = 1..256 (partners b = a+delta mod 512). Deltas 1..255
  cover each unordered pair once; delta=256 pairs appear from both
  endpoints, and each endpoint keeps its own copy in its row sum while
  the cross path scatters only delta=1..255, so no correction columns
  are needed. The self term exp(0)=1 is added on the host.

  Identity: |u-v| = 2*relu(u-v) - u + v  =>  dist = 2R' - S_b + S_a with
  R' = sum_d relu(x_b - x_a), S = sum_d x. PSUM accumulates
  (R' - S_b/2) per row (4 ones-matmuls + one -I matmul with the halved
  S16 window), so one BIAS-FREE activation computes
      P = exp(-2*psum) = exp(S_b - 2R') = exp(S_a - dist)
  batched over FOUR rows at a time ([128,1024] spanning 2 psum banks,
  260ns/row vs v2's 585ns/row exp+accum). P can reach exp(+17) so P8 is
  stored in BF16 (8-bit exponent - no overflow; 0.4% mantissa error is
  well inside tolerance). A single DVE tensor_scalar rider per row then
  applies the row factor and the row sum in one 4x-mode op:
      dump = P * exp(-S_a)   (G scalar, f32),  accum_out = sum(dump).

  Per row j:
    Pool: relu chunk 0 (ready earliest from the projection), 213ns
    DVE : relu chunks 1,2,3 (fp16 4x mode, 127ns each)
    PE  : 4 d-sum matmuls + 1 negI(S16h window) matmul -> psum
    ACT : exp half-group [128,1024] -> P8 bf16 (at rows 3 and 7 of 8)
    DVE : rider ts: dump = P*G, accum_out -> raw32[:, j]   (127ns)
    Pool: cross[k, j+1..j+255] += dump[k, 0:255]           (213ns)
  Riders lag one half-group behind the front so the DVE queue never
  waits on an exp; crosses follow one row behind riders.

  dist psum layout: partition 32c+m holds k=25c+m (m<25); host
  reassembles own rows from raw32 and scatters cross columns t=1..318
  to rows (64c+t) % 512.
"""

import sys
import numpy as np

for _p in ("/opt/trn_rl_repo",):
    if _p not in sys.path:
        sys.path.insert(0, _p)

B = 512
F = 1024
K = 100
D = 5
KD = K * D  # 500
NCORES = 8
JPC = B // NCORES  # 64 output rows per core
NCHUNK = 4  # kd chunks of 125
CHUNK = KD // NCHUNK  # 125
KPC = K // NCHUNK  # 25 k's per chunk
FD = 256  # per-row window: delta = 1..256
W = JPC + FD  # 320 columns of x needed per core
GROUP = 8  # rows per psum wide tile
HALF = GROUP // 2  # rows per exp batch
LAG = 2 * HALF  # rider emission lag (rows)

_NC_CACHE = {}


def build_nc():
    import contextlib

    import concourse.bass as bass
    import concourse.bacc as bacc
    import concourse.mybir as mybir
    from concourse.tile import TileContext

    nc = bacc.Bacc(None, target_bir_lowering=False, debug=True)

    inT = nc.declare_dram_parameter("inT", [F, W], mybir.dt.float16, isOutput=False)
    Tm = nc.declare_dram_parameter("Tm", [F, KD], mybir.dt.float16, isOutput=False)
    onesd = nc.declare_dram_parameter(
        "onesd", [CHUNK, 32], mybir.dt.float16, isOutput=False
    )
    negI = nc.declare_dram_parameter("negI", [128, 128], mybir.dt.float16, isOutput=False)
    raw_out = nc.declare_dram_parameter(
        "raw", [128, JPC], mybir.dt.float32, isOutput=True
    )
    cross_out = nc.declare_dram_parameter(
        "cross", [128, W], mybir.dt.float32, isOutput=True
    )

    with TileContext(nc) as tc:
        with tc.tile_pool(name="persist", bufs=1) as pp:
            T_sb = pp.tile([128, 8 * KD], mybir.dt.float16, name="T_sb")
            inT_sb = pp.tile([128, 8 * W], mybir.dt.float16, name="inT_sb")
            ones_sb = pp.tile([CHUNK, 32], mybir.dt.float16, name="ones_sb")
            negI_sb = pp.tile([128, 128], mybir.dt.float16, name="negI_sb")
            xT_sb = pp.tile([128, NCHUNK * W], mybir.dt.float16, name="xT_sb")
            # f32 upcasts of xT columns 0..JPC (tensor_scalar per-partition
            # scalars must be f32)
            xTj_sb = pp.tile([128, NCHUNK * JPC], mybir.dt.float32, name="xTj_sb")
            S16h_sb = pp.tile([128, W], mybir.dt.float16, name="S16h_sb")
            E_sb = pp.tile([128, W], mybir.dt.bfloat16, name="E_sb")
            G_sb = pp.tile([128, JPC], mybir.dt.float32, name="G_sb")
            cross_sb = pp.tile([128, W], mybir.dt.float32, name="cross_sb")
            raw_sb = pp.tile([128, JPC], mybir.dt.float32, name="raw_sb")
            NP8 = 6
            p8_bufs = [
                pp.tile([128, HALF * FD], mybir.dt.bfloat16, name=f"p8_{i}")
                for i in range(NP8)
            ]
            pe4_bufs = [
                pp.tile([128, HALF * FD], mybir.dt.bfloat16, name=f"pe4_{i}")
                for i in range(NP8)
            ]
            NAB = 48
            ab_bufs = [
                pp.tile([CHUNK, FD], mybir.dt.float16, name=f"ab{i}")
                for i in range(NAB)
            ]
            NDP = 8
            dump_bufs = [
                pp.tile([128, FD], mybir.dt.float16, name=f"dp{i}")
                for i in range(NDP)
            ]

            # warm the ACT exp table while DMAs run (table load ~1.3us)
            warm_sb = pp.tile([1, 1], mybir.dt.float32, name="warm_sb")
            nc.vector.memset(warm_sb[:, :], 0.0)
            nc.scalar.activation(
                warm_sb[:, :], warm_sb[:, :], mybir.ActivationFunctionType.Exp
            )
            nc.vector.memset(cross_sb[:, :], 0.0)

            # --- load inputs: T quarters on the SP queue, inT quarters on
            # the ACT queue so descriptor generation overlaps ---
            for h in range(4):
                nc.sync.dma_start(
                    out=T_sb[:, h * 2 * KD : (h + 1) * 2 * KD],
                    in_=Tm[h * 256 : (h + 1) * 256, :].rearrange(
                        "(t p) c -> p t c", t=2
                    ),
                )
                nc.scalar.dma_start(
                    out=inT_sb[:, h * 2 * W : (h + 1) * 2 * W],
                    in_=inT[h * 256 : (h + 1) * 256, :].rearrange(
                        "(t p) c -> p t c", t=2
                    ),
                )
            nc.sync.dma_start(out=ones_sb[:, :], in_=onesd[:, :])
            nc.sync.dma_start(out=negI_sb[:, :], in_=negI[:, :])

            psum_es = contextlib.ExitStack()
            psum = psum_es.enter_context(
                tc.tile_pool(name="psum", bufs=1, space="PSUM")
            )
            # Four 2-bank-wide dist tiles (all 8 psum banks), one per
            # half-group so WAR tracking decouples each exp from the next
            # halves' matmuls. The projection aliases chunk c's accumulator
            # into wide[c]; S goes into wide[3]'s second bank.
            wide = [
                psum.tile([128, HALF * FD], mybir.dt.float32, name=f"wide{i}")
                for i in range(4)
            ]
            xt_ps = [wide[c][0:CHUNK, 0:W] for c in range(NCHUNK)]
            S_ps = wide[3][:, 512 : 512 + W]
            # projection: t-outer for tiles 0..5 (runnable as DMA quarters
            # land), then per-chunk tails so each chunk's psum->sbuf copy
            # starts while the next chunk's tail matmuls run
            for t in range(6):
                for c in range(NCHUNK):
                    nc.tensor.matmul(
                        xt_ps[c],
                        T_sb[:, t * KD + c * CHUNK : t * KD + (c + 1) * CHUNK],
                        inT_sb[:, t * W : (t + 1) * W],
                        start=(t == 0),
                        stop=False,
                        skip_group_check=True,
                    )
            for c in range(NCHUNK):
                for t in (6, 7):
                    nc.tensor.matmul(
                        xt_ps[c],
                        T_sb[:, t * KD + c * CHUNK : t * KD + (c + 1) * CHUNK],
                        inT_sb[:, t * W : (t + 1) * W],
                        start=False,
                        stop=(t == 7),
                        skip_group_check=True,
                    )
                # psum->sbuf copies split between DVE and ACT
                eng = nc.vector.tensor_copy if c % 2 == 0 else nc.scalar.copy
                eng(xT_sb[0:CHUNK, c * W : (c + 1) * W], xt_ps[c])
                if c % 2 == 0:
                    nc.vector.tensor_copy(
                        xTj_sb[0:CHUNK, c * JPC : (c + 1) * JPC],
                        xT_sb[0:CHUNK, c * W : c * W + JPC],
                    )
                else:
                    nc.scalar.copy(
                        xTj_sb[0:CHUNK, c * JPC : (c + 1) * JPC],
                        xT_sb[0:CHUNK, c * W : c * W + JPC],
                    )
                # S[25c+m at partition 32c+m, i] = sum_d x[kd, i]
                nc.tensor.matmul(
                    S_ps[32 * c : 32 * c + 32, :],
                    ones_sb[:, :],
                    xT_sb[0:CHUNK, c * W : (c + 1) * W],
                    start=True,
                    stop=True,
                    tile_position=(0, 32 * c),
                )
            # S16h = S/2 (negI matmul's moving operand: psum gets -S_b/2 so
            # exp(-2*psum) carries exp(+S_b)); G = exp(-S) f32 row scalars
            nc.scalar.mul(S16h_sb[:, :], S_ps[:, :], 0.5)
            nc.scalar.activation(
                E_sb[:, :], S_ps[:, :], mybir.ActivationFunctionType.Exp,
                bias=0.0, scale=1.0,
            )
            nc.scalar.activation(
                G_sb[:, :], S_ps[:, 0:JPC], mybir.ActivationFunctionType.Exp,
                bias=0.0, scale=-1.0,
            )

            import bass_rust

            # --- main loop over output rows ---
            def emit_front(j):
                hh = j // HALF
                jj = j % HALF
                dist = wide[hh % 4]
                for c in range(NCHUNK):
                    ab = ab_bufs[(j * NCHUNK + c) % NAB]
                    # Pool takes chunk 0 (ready earliest) plus chunk 1 on
                    # every fourth row; DVE the rest
                    on_pool = c == 0 or (c == 1 and j % 4 == 1)
                    eng = nc.gpsimd if on_pool else nc.vector
                    eng.tensor_scalar(
                        ab[:, :],
                        xT_sb[0:CHUNK, c * W + j + 1 : c * W + j + 1 + FD],
                        xTj_sb[0:CHUNK, c * JPC + j : c * JPC + j + 1],
                        0.0,
                        mybir.AluOpType.subtract,
                        mybir.AluOpType.max,
                    )
                    nc.tensor.matmul(
                        dist[32 * c : 32 * c + 32, jj * FD : (jj + 1) * FD],
                        ones_sb[:, :],
                        ab[:, :],
                        start=True,
                        stop=(hh % 2 == 1),
                        tile_position=(0, 32 * c),
                        skip_group_check=True,
                    )
                if hh % 2 == 0:
                    # negI path: psum -= S16h[., j+1 : j+257] (closes groups)
                    nc.tensor.matmul(
                        dist[:, jj * FD : (jj + 1) * FD],
                        negI_sb[:, :],
                        S16h_sb[:, j + 1 : j + 1 + FD],
                        start=False,
                        stop=True,
                        skip_group_check=True,
                    )
                if jj == HALF - 1:
                    # batched P = exp(-2*psum) over a 4-row half-group
                    nc.scalar.activation(
                        p8_bufs[hh % NP8][:, :],
                        dist[:, :],
                        mybir.ActivationFunctionType.Exp,
                        bias=0.0,
                        scale=-2.0,
                    )
                    if hh % 2 == 1:
                        # E path: one DVE 2x multiply applies exp(S_b) to the
                        # whole half via an overlapping-window AP over E
                        ewin = E_sb[:, HALF * hh + 1 : HALF * hh + 1 + FD].copy()
                        ewin.ap = bass_rust.VecI64Pair(
                            [tuple(ewin.ap[0]), (1, HALF), (1, FD)]
                        )
                        nc.vector.tensor_tensor(
                            pe4_bufs[hh % NP8][:, :].rearrange(
                                "p (r c) -> p r c", r=HALF
                            ),
                            p8_bufs[hh % NP8][:, :].rearrange(
                                "p (r c) -> p r c", r=HALF
                            ),
                            ewin,
                            mybir.AluOpType.mult,
                        )

            def emit_rider(j):
                hg = j // HALF
                jj = j % HALF
                src = p8_bufs if hg % 2 == 0 else pe4_bufs
                p8 = src[hg % NP8]
                # dump = P * exp(-S_a);  accum_out = row sum (free on 4x)
                nc.vector.tensor_scalar(
                    dump_bufs[j % NDP][:, :],
                    p8[:, jj * FD : (jj + 1) * FD],
                    G_sb[:, j : j + 1],
                    0.0,
                    mybir.AluOpType.mult,
                    mybir.AluOpType.add,
                    accum_out=raw_sb[:, j : j + 1],
                )

            def emit_cross(j):
                # cross[k, j+delta] += dump[k, delta-1] for delta = 1..255
                # (delta=256 belongs to the partner row's own sum)
                nc.gpsimd.tensor_tensor(
                    cross_sb[:, j + 1 : j + FD],
                    cross_sb[:, j + 1 : j + FD],
                    dump_bufs[j % NDP][:, 0 : FD - 1],
                    mybir.AluOpType.add,
                )

            for j in range(JPC + LAG + 1):
                if j < JPC:
                    emit_front(j)
                if LAG <= j < JPC + LAG:
                    emit_rider(j - LAG)
                if j > LAG:
                    emit_cross(j - LAG - 1)

            psum_es.close()
            nc.scalar.dma_start(out=raw_out[:, :], in_=raw_sb[:, :])
            nc.sync.dma_start(out=cross_out[:, :], in_=cross_sb[:, :])

    nc.finalize()
    return nc


def _aux_consts():
    ob = np.zeros([CHUNK, 32], dtype=np.float16)
    for m in range(KPC):
        ob[5 * m : 5 * m + 5, m] = 1.0
    negI = (-np.eye(128)).astype(np.float16)
    return ob, negI


def make_in_maps(inputs, T):
    f16 = np.float16
    Tm = np.asarray(T, dtype=np.float32).astype(f16)
    ob, negI = _aux_consts()
    in_maps = []
    for c in range(NCORES):
        rolled = np.roll(np.asarray(inputs, dtype=np.float32), -JPC * c, axis=0)
        inTc = np.ascontiguousarray(rolled[0:W].T).astype(f16)
        in_maps.append(
            {
                "inT": inTc,
                "Tm": Tm,
                "onesd": ob,
                "negI": negI,
            }
        )
    return in_maps


def assemble_output(results):
    out = np.zeros([B, K], dtype=np.float32)
    for c in range(NCORES):
        rawc = np.asarray(results[c]["raw"], dtype=np.float32)  # [128, JPC]
        cross = np.asarray(results[c]["cross"], dtype=np.float32)  # [128, W]
        for cc in range(NCHUNK):
            ksl = slice(32 * cc, 32 * cc + KPC)
            kg = slice(KPC * cc, KPC * (cc + 1))
            # own rows: global rows 64c..64c+63 (+1.0 self term)
            out[JPC * c : JPC * (c + 1), kg] += rawc[ksl, :].T + 1.0
            # cross rows: global rows (64c + t) % 512 for t = 1..W-1
            rows = (JPC * c + np.arange(1, W)) % B
            np.add.at(
                out,
                (rows[:, None], np.arange(KPC * cc, KPC * (cc + 1))[None, :]),
                cross[ksl, 1:W].T,
            )
    return out


def kernel(inputs, T):
    from concourse.bass_utils import run_bass_kernel_spmd

    if "nc" not in _NC_CACHE:
        _NC_CACHE["nc"] = build_nc()
    nc = _NC_CACHE["nc"]
    in_maps = make_in_maps(inputs, T)
    res = run_bass_kernel_spmd(nc, in_maps, list(range(NCORES)))
    return assemble_output(res.results)


if __name__ == "__main__":
    sys.path.insert(0, "/root/problem")
    from reference import setup_inputs, reference

    inputs = setup_inputs()
    expected = np.asarray(reference(**inputs))
    actual = kernel(**{k: np.asarray(v) for k, v in inputs.items()})
    err = np.abs(actual - expected)
    rel = np.linalg.norm(actual - expected) / np.linalg.norm(expected)
    print(f"max abs err: {err.max():.3e}")
    print(f"Relative error: {rel:.3e}")


# revision 19
# speedup vs baseline: 1.3233x; 1.0422x over previous
"""
MinibatchDiscrimination kernel for 8x TRN2 NeuronCores (Bass/Tile).

Math:  x = inputs @ T  -> [B, K, D] with B=512, K=100, D=5
       out[a,k] = sum_b exp(-sum_d |x[a,k,d]-x[b,k,d]|)

v4 strategy (v2 = 49.8us, v3 experiments showed per-row ACT exp overhead
and batched-multiply chains were the limiters):

  Pair coverage (as v2): core c owns global rows a = 64c+j (j=0..63) and
  window delta = 1..256 (partners b = a+delta mod 512). Deltas 1..255
  cover each unordered pair once; delta=256 pairs appear from both
  endpoints, and each endpoint keeps its own copy in its row sum while
  the cross path scatters only delta=1..255, so no correction columns
  are needed. The self term exp(0)=1 is added on the host.

  Identity: |u-v| = 2*relu(u-v) - u + v  =>  dist = 2R' - S_b + S_a with
  R' = sum_d relu(x_b - x_a), S = sum_d x. PSUM accumulates
  (R' - S_b/2) per row (4 ones-matmuls + one -I matmul with the halved
  S16 window), so one BIAS-FREE activation computes
      P = exp(-2*psum) = exp(S_b - 2R') = exp(S_a - dist)
  batched over FOUR rows at a time ([128,1024] spanning 2 psum banks,
  260ns/row vs v2's 585ns/row exp+accum). P can reach exp(+17) so P8 is
  stored in BF16 (8-bit exponent - no overflow; 0.4% mantissa error is
  well inside tolerance). A single DVE tensor_scalar rider per row then
  applies the row factor and the row sum in one 4x-mode op:
      dump = P * exp(-S_a)   (G scalar, f32),  accum_out = sum(dump).

  Per row j:
    Pool: relu chunk 0 (ready earliest from the projection), 213ns
    DVE : relu chunks 1,2,3 (fp16 4x mode, 127ns each)
    PE  : 4 d-sum matmuls + 1 negI(S16h window) matmul -> psum
    ACT : exp half-group [128,1024] -> P8 bf16 (at rows 3 and 7 of 8)
    DVE : rider ts: dump = P*G, accum_out -> raw32[:, j]   (127ns)
    Pool: cross[k, j+1..j+255] += dump[k, 0:255]           (213ns)
  Riders lag one half-group behind the front so the DVE queue never
  waits on an exp; crosses follow one row behind riders.

  dist psum layout: partition 32c+m holds k=25c+m (m<25); host
  reassembles own rows from raw32 and scatters cross columns t=1..318
  to rows (64c+t) % 512.
"""

import sys
import numpy as np

for _p in ("/opt/trn_rl_repo",):
    if _p not in sys.path:
        sys.path.insert(0, _p)

B = 512
F = 1024
K = 100
D = 5
KD = K * D  # 500
NCORES = 8
JPC = B // NCORES  # 64 output rows per core
NCHUNK = 4  # kd chunks of 125
CHUNK = KD // NCHUNK  # 125
KPC = K // NCHUNK  # 25 k's per chunk
FD = 256  # per-row window: delta = 1..256
W = JPC + FD  # 320 columns of x needed per core
GROUP = 8  # rows per psum wide tile
HALF = GROUP // 2  # rows per exp batch
LAG = 2 * HALF  # rider emission lag (rows)

_NC_CACHE = {}


def build_nc():
    import contextlib

    import concourse.bass as bass
    import concourse.bacc as bacc
    import concourse.mybir as mybir
    from concourse.tile import TileContext

    nc = bacc.Bacc(None, target_bir_lowering=False, debug=True)

    inT = nc.declare_dram_parameter("inT", [F, W], mybir.dt.float16, isOutput=False)
    Tm = nc.declare_dram_parameter("Tm", [F, KD], mybir.dt.float16, isOutput=False)
    onesd = nc.declare_dram_parameter(
        "onesd", [CHUNK, 32], mybir.dt.float16, isOutput=False
    )
    negI = nc.declare_dram_parameter("negI", [128, 128], mybir.dt.float16, isOutput=False)
    raw_out = nc.declare_dram_parameter(
        "raw", [128, JPC], mybir.dt.float32, isOutput=True
    )
    cross_out = nc.declare_dram_parameter(
        "cross", [128, W], mybir.dt.float32, isOutput=True
    )

    with TileContext(nc) as tc:
        with tc.tile_pool(name="persist", bufs=1) as pp:
            T_sb = pp.tile([128, 8 * KD], mybir.dt.float16, name="T_sb")
            inT_sb = pp.tile([128, 8 * W], mybir.dt.float16, name="inT_sb")
            ones_sb = pp.tile([CHUNK, 32], mybir.dt.float16, name="ones_sb")
            negI_sb = pp.tile([128, 128], mybir.dt.float16, name="negI_sb")
            xT_sb = pp.tile([128, NCHUNK * W], mybir.dt.float16, name="xT_sb")
            # f32 upcasts of xT columns 0..JPC (tensor_scalar per-partition
            # scalars must be f32)
            xTj_sb = pp.tile([128, NCHUNK * JPC], mybir.dt.float32, name="xTj_sb")
            S16h_sb = pp.tile([128, W], mybir.dt.float16, name="S16h_sb")
            E_sb = pp.tile([128, W], mybir.dt.bfloat16, name="E_sb")
            G_sb = pp.tile([128, JPC], mybir.dt.float32, name="G_sb")
            cross_sb = pp.tile([128, W], mybir.dt.float32, name="cross_sb")
            raw_sb = pp.tile([128, JPC], mybir.dt.float32, name="raw_sb")
            NP8 = 6
            p8_bufs = [
                pp.tile([128, HALF * FD], mybir.dt.bfloat16, name=f"p8_{i}")
                for i in range(NP8)
            ]
            pe4_bufs = [
                pp.tile([128, HALF * FD], mybir.dt.bfloat16, name=f"pe4_{i}")
                for i in range(NP8)
            ]
            NAB = 48
            ab_bufs = [
                pp.tile([CHUNK, FD], mybir.dt.float16, name=f"ab{i}")
                for i in range(NAB)
            ]
            NDP = 8
            dump_bufs = [
                pp.tile([128, FD], mybir.dt.float16, name=f"dp{i}")
                for i in range(NDP)
            ]

            # warm the ACT exp table while DMAs run (table load ~1.3us)
            warm_sb = pp.tile([1, 1], mybir.dt.float32, name="warm_sb")
            nc.vector.memset(warm_sb[:, :], 0.0)
            nc.scalar.activation(
                warm_sb[:, :], warm_sb[:, :], mybir.ActivationFunctionType.Exp
            )
            nc.vector.memset(cross_sb[:, :], 0.0)

            # --- load inputs: T quarters on the SP queue, inT quarters on
            # the ACT queue so descriptor generation overlaps ---
            for h in range(4):
                nc.sync.dma_start(
                    out=T_sb[:, h * 2 * KD : (h + 1) * 2 * KD],
                    in_=Tm[h * 256 : (h + 1) * 256, :].rearrange(
                        "(t p) c -> p t c", t=2
                    ),
                )
                nc.scalar.dma_start(
                    out=inT_sb[:, h * 2 * W : (h + 1) * 2 * W],
                    in_=inT[h * 256 : (h + 1) * 256, :].rearrange(
                        "(t p) c -> p t c", t=2
                    ),
                )
            nc.sync.dma_start(out=ones_sb[:, :], in_=onesd[:, :])
            nc.sync.dma_start(out=negI_sb[:, :], in_=negI[:, :])

            psum_es = contextlib.ExitStack()
            psum = psum_es.enter_context(
                tc.tile_pool(name="psum", bufs=1, space="PSUM")
            )
            # Four 2-bank-wide dist tiles (all 8 psum banks), one per
            # half-group so WAR tracking decouples each exp from the next
            # halves' matmuls. The projection aliases chunk c's accumulator
            # into wide[c]; S goes into wide[3]'s second bank.
            wide = [
                psum.tile([128, HALF * FD], mybir.dt.float32, name=f"wide{i}")
                for i in range(4)
            ]
            xt_ps = [wide[c][0:CHUNK, 0:W] for c in range(NCHUNK)]
            S_ps = wide[3][:, 512 : 512 + W]
            # projection: t-outer for tiles 0..5 (runnable as DMA quarters
            # land), then per-chunk tails so each chunk's psum->sbuf copy
            # starts while the next chunk's tail matmuls run
            for t in range(6):
                for c in range(NCHUNK):
                    nc.tensor.matmul(
                        xt_ps[c],
                        T_sb[:, t * KD + c * CHUNK : t * KD + (c + 1) * CHUNK],
                        inT_sb[:, t * W : (t + 1) * W],
                        start=(t == 0),
                        stop=False,
                        skip_group_check=True,
                    )
            for c in range(NCHUNK):
                for t in (6, 7):
                    nc.tensor.matmul(
                        xt_ps[c],
                        T_sb[:, t * KD + c * CHUNK : t * KD + (c + 1) * CHUNK],
                        inT_sb[:, t * W : (t + 1) * W],
                        start=False,
                        stop=(t == 7),
                        skip_group_check=True,
                    )
                # psum->sbuf copies split between DVE and ACT
                eng = nc.vector.tensor_copy if c % 2 == 0 else nc.scalar.copy
                eng(xT_sb[0:CHUNK, c * W : (c + 1) * W], xt_ps[c])
                if c % 2 == 0:
                    nc.vector.tensor_copy(
                        xTj_sb[0:CHUNK, c * JPC : (c + 1) * JPC],
                        xT_sb[0:CHUNK, c * W : c * W + JPC],
                    )
                else:
                    nc.scalar.copy(
                        xTj_sb[0:CHUNK, c * JPC : (c + 1) * JPC],
                        xT_sb[0:CHUNK, c * W : c * W + JPC],
                    )
                # S[25c+m at partition 32c+m, i] = sum_d x[kd, i]
                nc.tensor.matmul(
                    S_ps[32 * c : 32 * c + 32, :],
                    ones_sb[:, :],
                    xT_sb[0:CHUNK, c * W : (c + 1) * W],
                    start=True,
                    stop=True,
                    tile_position=(0, 32 * c),
                )
            # S16h = S/2 (negI matmul's moving operand: psum gets -S_b/2 so
            # exp(-2*psum) carries exp(+S_b)); G = exp(-S) f32 row scalars
            nc.scalar.mul(S16h_sb[:, :], S_ps[:, :], 0.5)
            nc.scalar.activation(
                E_sb[:, :], S_ps[:, :], mybir.ActivationFunctionType.Exp,
                bias=0.0, scale=1.0,
            )
            nc.scalar.activation(
                G_sb[:, :], S_ps[:, 0:JPC], mybir.ActivationFunctionType.Exp,
                bias=0.0, scale=-1.0,
            )

            import bass_rust

            # --- main loop over output rows ---
            def emit_front(j):
                hh = j // HALF
                jj = j % HALF
                dist = wide[hh % 4]
                for c in range(NCHUNK):
                    ab = ab_bufs[(j * NCHUNK + c) % NAB]
                    # Pool takes chunk 0 (ready earliest) plus chunk 1 on
                    # every fourth row; DVE the rest
                    on_pool = c == 0 or (c == 1 and j % 2 == 1)
                    eng = nc.gpsimd if on_pool else nc.vector
                    eng.tensor_scalar(
                        ab[:, :],
                        xT_sb[0:CHUNK, c * W + j + 1 : c * W + j + 1 + FD],
                        xTj_sb[0:CHUNK, c * JPC + j : c * JPC + j + 1],
                        0.0,
                        mybir.AluOpType.subtract,
                        mybir.AluOpType.max,
                    )
                    nc.tensor.matmul(
                        dist[32 * c : 32 * c + 32, jj * FD : (jj + 1) * FD],
                        ones_sb[:, :],
                        ab[:, :],
                        start=True,
                        stop=(hh % 2 == 0),
                        tile_position=(0, 32 * c),
                        skip_group_check=True,
                    )
                if hh % 2 == 1:
                    # negI path: psum -= S16h[., j+1 : j+257] (closes groups)
                    nc.tensor.matmul(
                        dist[:, jj * FD : (jj + 1) * FD],
                        negI_sb[:, :],
                        S16h_sb[:, j + 1 : j + 1 + FD],
                        start=False,
                        stop=True,
                        skip_group_check=True,
                    )
                if jj == HALF - 1:
                    # batched P = exp(-2*psum) over a 4-row half-group
                    nc.scalar.activation(
                        p8_bufs[hh % NP8][:, :],
                        dist[:, :],
                        mybir.ActivationFunctionType.Exp,
                        bias=0.0,
                        scale=-2.0,
                    )
                    if hh % 2 == 0:
                        # E path: one DVE 2x multiply applies exp(S_b) to the
                        # whole half via an overlapping-window AP over E
                        ewin = E_sb[:, HALF * hh + 1 : HALF * hh + 1 + FD].copy()
                        ewin.ap = bass_rust.VecI64Pair(
                            [tuple(ewin.ap[0]), (1, HALF), (1, FD)]
                        )
                        nc.vector.tensor_tensor(
                            pe4_bufs[hh % NP8][:, :].rearrange(
                                "p (r c) -> p r c", r=HALF
                            ),
                            p8_bufs[hh % NP8][:, :].rearrange(
                                "p (r c) -> p r c", r=HALF
                            ),
                            ewin,
                            mybir.AluOpType.mult,
                        )

            def emit_rider(j):
                hg = j // HALF
                jj = j % HALF
                src = p8_bufs if hg % 2 == 1 else pe4_bufs
                p8 = src[hg % NP8]
                # dump = P * exp(-S_a);  accum_out = row sum (free on 4x)
                nc.vector.tensor_scalar(
                    dump_bufs[j % NDP][:, :],
                    p8[:, jj * FD : (jj + 1) * FD],
                    G_sb[:, j : j + 1],
                    0.0,
                    mybir.AluOpType.mult,
                    mybir.AluOpType.add,
                    accum_out=raw_sb[:, j : j + 1],
                )

            def emit_cross(j):
                # cross[k, j+delta] += dump[k, delta-1] for delta = 1..255
                # (delta=256 belongs to the partner row's own sum)
                nc.gpsimd.tensor_tensor(
                    cross_sb[:, j + 1 : j + FD],
                    cross_sb[:, j + 1 : j + FD],
                    dump_bufs[j % NDP][:, 0 : FD - 1],
                    mybir.AluOpType.add,
                )

            for j in range(JPC + LAG + 1):
                if j < JPC:
                    emit_front(j)
                if LAG <= j < JPC + LAG:
                    emit_rider(j - LAG)
                if j > LAG:
                    emit_cross(j - LAG - 1)
                if j - LAG == 47:
                    nc.scalar.dma_start(
                        out=raw_out[:, 0:48], in_=raw_sb[:, 0:48]
                    )

            psum_es.close()
            nc.scalar.dma_start(out=raw_out[:, 48:JPC], in_=raw_sb[:, 48:JPC])
            nc.sync.dma_start(out=cross_out[:, :], in_=cross_sb[:, :])

    nc.finalize()
    return nc


def _aux_consts():
    ob = np.zeros([CHUNK, 32], dtype=np.float16)
    for m in range(KPC):
        ob[5 * m : 5 * m + 5, m] = 1.0
    negI = (-np.eye(128)).astype(np.float16)
    return ob, negI


def make_in_maps(inputs, T):
    f16 = np.float16
    Tm = np.asarray(T, dtype=np.float32).astype(f16)
    ob, negI = _aux_consts()
    in_maps = []
    for c in range(NCORES):
        rolled = np.roll(np.asarray(inputs, dtype=np.float32), -JPC * c, axis=0)
        inTc = np.ascontiguousarray(rolled[0:W].T).astype(f16)
        in_maps.append(
            {
                "inT": inTc,
                "Tm": Tm,
                "onesd": ob,
                "negI": negI,
            }
        )
    return in_maps


def assemble_output(results):
    out = np.zeros([B, K], dtype=np.float32)
    for c in range(NCORES):
        rawc = np.asarray(results[c]["raw"], dtype=np.float32)  # [128, JPC]
        cross = np.asarray(results[c]["cross"], dtype=np.float32)  # [128, W]
        for cc in range(NCHUNK):
            ksl = slice(32 * cc, 32 * cc + KPC)
            kg = slice(KPC * cc, KPC * (cc + 1))
            # own rows: global rows 64c..64c+63 (+1.0 self term)
            out[JPC * c : JPC * (c + 1), kg] += rawc[ksl, :].T + 1.0
            # cross rows: global rows (64c + t) % 512 for t = 1..W-1
            rows = (JPC * c + np.arange(1, W)) % B
            np.add.at(
                out,
                (rows[:, None], np.arange(KPC * cc, KPC * (cc + 1))[None, :]),
                cross[ksl, 1:W].T,
            )
    return out


def kernel(inputs, T):
    from concourse.bass_utils import run_bass_kernel_spmd

    if "nc" not in _NC_CACHE:
        _NC_CACHE["nc"] = build_nc()
    nc = _NC_CACHE["nc"]
    in_maps = make_in_maps(inputs, T)
    res = run_bass_kernel_spmd(nc, in_maps, list(range(NCORES)))
    return assemble_output(res.results)


if __name__ == "__main__":
    sys.path.insert(0, "/root/problem")
    from reference import setup_inputs, reference

    inputs = setup_inputs()
    expected = np.asarray(reference(**inputs))
    actual = kernel(**{k: np.asarray(v) for k, v in inputs.items()})
    err = np.abs(actual - expected)
    rel = np.linalg.norm(actual - expected) / np.linalg.norm(expected)
    print(f"max abs err: {err.max():.3e}")
    print(f"Relative error: {rel:.3e}")


# revision 20
# speedup vs baseline: 1.3336x; 1.0078x over previous
"""
MinibatchDiscrimination kernel for 8x TRN2 NeuronCores (Bass/Tile).

Math:  x = inputs @ T  -> [B, K, D] with B=512, K=100, D=5
       out[a,k] = sum_b exp(-sum_d |x[a,k,d]-x[b,k,d]|)

v4 strategy (v2 = 49.8us, v3 experiments showed per-row ACT exp overhead
and batched-multiply chains were the limiters):

  Pair coverage (as v2): core c owns global rows a = 64c+j (j=0..63) and
  window delta = 1..256 (partners b = a+delta mod 512). Deltas 1..255
  cover each unordered pair once; delta=256 pairs appear from both
  endpoints, and each endpoint keeps its own copy in its row sum while
  the cross path scatters only delta=1..255, so no correction columns
  are needed. The self term exp(0)=1 is added on the host.

  Identity: |u-v| = 2*relu(u-v) - u + v  =>  dist = 2R' - S_b + S_a with
  R' = sum_d relu(x_b - x_a), S = sum_d x. PSUM accumulates
  (R' - S_b/2) per row (4 ones-matmuls + one -I matmul with the halved
  S16 window), so one BIAS-FREE activation computes
      P = exp(-2*psum) = exp(S_b - 2R') = exp(S_a - dist)
  batched over FOUR rows at a time ([128,1024] spanning 2 psum banks,
  260ns/row vs v2's 585ns/row exp+accum). P can reach exp(+17) so P8 is
  stored in BF16 (8-bit exponent - no overflow; 0.4% mantissa error is
  well inside tolerance). A single DVE tensor_scalar rider per row then
  applies the row factor and the row sum in one 4x-mode op:
      dump = P * exp(-S_a)   (G scalar, f32),  accum_out = sum(dump).

  Per row j:
    Pool: relu chunk 0 (ready earliest from the projection), 213ns
    DVE : relu chunks 1,2,3 (fp16 4x mode, 127ns each)
    PE  : 4 d-sum matmuls + 1 negI(S16h window) matmul -> psum
    ACT : exp half-group [128,1024] -> P8 bf16 (at rows 3 and 7 of 8)
    DVE : rider ts: dump = P*G, accum_out -> raw32[:, j]   (127ns)
    Pool: cross[k, j+1..j+255] += dump[k, 0:255]           (213ns)
  Riders lag one half-group behind the front so the DVE queue never
  waits on an exp; crosses follow one row behind riders.

  dist psum layout: partition 32c+m holds k=25c+m (m<25); host
  reassembles own rows from raw32 and scatters cross columns t=1..318
  to rows (64c+t) % 512.
"""

import sys
import numpy as np

for _p in ("/opt/trn_rl_repo",):
    if _p not in sys.path:
        sys.path.insert(0, _p)

B = 512
F = 1024
K = 100
D = 5
KD = K * D  # 500
NCORES = 8
JPC = B // NCORES  # 64 output rows per core
NCHUNK = 4  # kd chunks of 125
CHUNK = KD // NCHUNK  # 125
KPC = K // NCHUNK  # 25 k's per chunk
FD = 256  # per-row window: delta = 1..256
W = JPC + FD  # 320 columns of x needed per core
GROUP = 8  # rows per psum wide tile
HALF = GROUP // 2  # rows per exp batch
LAG = 2 * HALF  # rider emission lag (rows)

_NC_CACHE = {}


def build_nc():
    import contextlib

    import concourse.bass as bass
    import concourse.bacc as bacc
    import concourse.mybir as mybir
    from concourse.tile import TileContext

    nc = bacc.Bacc(None, target_bir_lowering=False, debug=True)

    inT = nc.declare_dram_parameter("inT", [F, W], mybir.dt.float16, isOutput=False)
    Tm = nc.declare_dram_parameter("Tm", [F, KD], mybir.dt.float16, isOutput=False)
    onesd = nc.declare_dram_parameter(
        "onesd", [CHUNK, 64], mybir.dt.float16, isOutput=False
    )
    negI = nc.declare_dram_parameter("negI", [128, 128], mybir.dt.float16, isOutput=False)
    raw_out = nc.declare_dram_parameter(
        "raw", [128, JPC], mybir.dt.float32, isOutput=True
    )
    cross_out = nc.declare_dram_parameter(
        "cross", [128, W], mybir.dt.float32, isOutput=True
    )

    with TileContext(nc) as tc:
        with tc.tile_pool(name="persist", bufs=1) as pp:
            T_sb = pp.tile([128, 8 * KD], mybir.dt.float16, name="T_sb")
            inT_sb = pp.tile([128, 8 * W], mybir.dt.float16, name="inT_sb")
            ones_sb = pp.tile([CHUNK, 64], mybir.dt.float16, name="ones_sb")
            negI_sb = pp.tile([128, 128], mybir.dt.float16, name="negI_sb")
            xT_sb = pp.tile([128, NCHUNK * W], mybir.dt.float16, name="xT_sb")
            # f32 upcasts of xT columns 0..JPC (tensor_scalar per-partition
            # scalars must be f32)
            xTj_sb = pp.tile([128, NCHUNK * JPC], mybir.dt.float32, name="xTj_sb")
            S16h_sb = pp.tile([128, W], mybir.dt.float16, name="S16h_sb")
            E_sb = pp.tile([128, W], mybir.dt.bfloat16, name="E_sb")
            EB_sb = pp.tile([128, W], mybir.dt.bfloat16, name="EB_sb")
            G_sb = pp.tile([128, JPC], mybir.dt.float32, name="G_sb")
            GB_sb = pp.tile([128, JPC], mybir.dt.float32, name="GB_sb")
            negx3_sb = pp.tile([CHUNK, JPC], mybir.dt.float32, name="negx3_sb")
            cross_sb = pp.tile([128, W], mybir.dt.float32, name="cross_sb")
            raw_sb = pp.tile([128, JPC], mybir.dt.float32, name="raw_sb")
            NP8 = 6
            p8_bufs = [
                pp.tile([128, HALF * FD], mybir.dt.bfloat16, name=f"p8_{i}")
                for i in range(NP8)
            ]
            pe4_bufs = [
                pp.tile([128, HALF * FD], mybir.dt.bfloat16, name=f"pe4_{i}")
                for i in range(NP8)
            ]
            NAB = 48
            ab_bufs = [
                pp.tile([CHUNK, FD], mybir.dt.float16, name=f"ab{i}")
                for i in range(NAB)
            ]
            NAB3 = 16
            ab3_bufs = [
                pp.tile([CHUNK, FD], mybir.dt.float16, name=f"ab3_{i}")
                for i in range(NAB3)
            ]
            NDP = 8
            dump_bufs = [
                pp.tile([128, FD], mybir.dt.float16, name=f"dp{i}")
                for i in range(NDP)
            ]

            # warm the ACT exp table while DMAs run (table load ~1.3us)
            warm_sb = pp.tile([1, 1], mybir.dt.float32, name="warm_sb")
            nc.vector.memset(warm_sb[:, :], 0.0)
            nc.scalar.activation(
                warm_sb[:, :], warm_sb[:, :], mybir.ActivationFunctionType.Exp
            )
            nc.vector.memset(cross_sb[:, :], 0.0)

            # --- load inputs: T quarters on the SP queue, inT quarters on
            # the ACT queue so descriptor generation overlaps ---
            for h in range(4):
                nc.sync.dma_start(
                    out=T_sb[:, h * 2 * KD : (h + 1) * 2 * KD],
                    in_=Tm[h * 256 : (h + 1) * 256, :].rearrange(
                        "(t p) c -> p t c", t=2
                    ),
                )
                nc.scalar.dma_start(
                    out=inT_sb[:, h * 2 * W : (h + 1) * 2 * W],
                    in_=inT[h * 256 : (h + 1) * 256, :].rearrange(
                        "(t p) c -> p t c", t=2
                    ),
                )
            nc.sync.dma_start(out=ones_sb[:, :], in_=onesd[:, :])
            nc.sync.dma_start(out=negI_sb[:, :], in_=negI[:, :])

            psum_es = contextlib.ExitStack()
            psum = psum_es.enter_context(
                tc.tile_pool(name="psum", bufs=1, space="PSUM")
            )
            # Four 2-bank-wide dist tiles (all 8 psum banks), one per
            # half-group so WAR tracking decouples each exp from the next
            # halves' matmuls. The projection aliases chunk c's accumulator
            # into wide[c]; S goes into wide[3]'s second bank.
            wide = [
                psum.tile([128, HALF * FD], mybir.dt.float32, name=f"wide{i}")
                for i in range(4)
            ]
            xt_ps = [wide[c][0:CHUNK, 0:W] for c in range(NCHUNK)]
            S_ps = wide[3][:, 512 : 512 + W]
            # projection: t-outer for tiles 0..5 (runnable as DMA quarters
            # land), then per-chunk tails so each chunk's psum->sbuf copy
            # starts while the next chunk's tail matmuls run
            for t in range(6):
                for c in range(NCHUNK):
                    nc.tensor.matmul(
                        xt_ps[c],
                        T_sb[:, t * KD + c * CHUNK : t * KD + (c + 1) * CHUNK],
                        inT_sb[:, t * W : (t + 1) * W],
                        start=(t == 0),
                        stop=False,
                        skip_group_check=True,
                    )
            for c in range(NCHUNK):
                for t in (6, 7):
                    nc.tensor.matmul(
                        xt_ps[c],
                        T_sb[:, t * KD + c * CHUNK : t * KD + (c + 1) * CHUNK],
                        inT_sb[:, t * W : (t + 1) * W],
                        start=False,
                        stop=(t == 7),
                        skip_group_check=True,
                    )
                # psum->sbuf copies split between DVE and ACT
                eng = nc.vector.tensor_copy if c % 2 == 0 else nc.scalar.copy
                eng(xT_sb[0:CHUNK, c * W : (c + 1) * W], xt_ps[c])
                if c % 2 == 0:
                    nc.vector.tensor_copy(
                        xTj_sb[0:CHUNK, c * JPC : (c + 1) * JPC],
                        xT_sb[0:CHUNK, c * W : c * W + JPC],
                    )
                else:
                    nc.scalar.copy(
                        xTj_sb[0:CHUNK, c * JPC : (c + 1) * JPC],
                        xT_sb[0:CHUNK, c * W : c * W + JPC],
                    )
                # S[25c+m at partition 32c+m, i] = sum_d x[kd, i]
                nc.tensor.matmul(
                    S_ps[32 * c : 32 * c + 32, :],
                    ones_sb[:, 0:32],
                    xT_sb[0:CHUNK, c * W : (c + 1) * W],
                    start=True,
                    stop=True,
                    tile_position=(0, 32 * c),
                )
            # S16h = S/2 (negI matmul's moving operand: psum gets -S_b/2 so
            # exp(-2*psum) carries exp(+S_b)); G = exp(-S) f32 row scalars
            nc.scalar.mul(S16h_sb[:, :], S_ps[:, :], 0.5)
            nc.scalar.activation(
                E_sb[:, :], S_ps[:, :], mybir.ActivationFunctionType.Exp,
                bias=0.0, scale=1.0,
            )
            nc.scalar.activation(
                G_sb[:, :], S_ps[:, 0:JPC], mybir.ActivationFunctionType.Exp,
                bias=0.0, scale=-1.0,
            )
            # B variants for ACT-Abs halves: chunk-3 k's (partitions 96+)
            # need no S correction (their |z| lands exactly in psum)
            nc.vector.tensor_scalar(
                negx3_sb[:, :], xTj_sb[0:CHUNK, 3 * JPC : 4 * JPC], -1.0, 0.0,
                mybir.AluOpType.mult, mybir.AluOpType.add,
            )
            nc.vector.tensor_copy(EB_sb[:, :], E_sb[:, :])
            nc.vector.memset(EB_sb[96:128, :], 1.0)
            nc.vector.tensor_copy(GB_sb[:, :], G_sb[:, :])
            nc.vector.memset(GB_sb[96:128, :], 1.0)

            import bass_rust

            def is_ehalf(hh):
                return hh % 2 == 0

            def emit_abs3(j):
                # ACT absorbs chunk 3 of E-half rows: |x_win - x_j| written
                # a group ahead so PE never waits on the ACT queue
                nc.scalar.activation(
                    ab3_bufs[j % NAB3][:, :],
                    xT_sb[0:CHUNK, 3 * W + j + 1 : 3 * W + j + 1 + FD],
                    mybir.ActivationFunctionType.Abs,
                    bias=negx3_sb[:, j : j + 1],
                    scale=1.0,
                )

            # --- main loop over output rows ---
            def emit_front(j):
                hh = j // HALF
                jj = j % HALF
                dist = wide[hh % 4]
                abs3 = is_ehalf(hh)
                for c in range(NCHUNK):
                    if c == 3 and abs3:
                        ab = ab3_bufs[j % NAB3]
                    else:
                        ab = ab_bufs[(j * NCHUNK + c) % NAB]
                        # Pool takes chunk 0 (ready earliest) plus chunk 1 on
                        # every other row; DVE the rest
                        on_pool = c == 0 or (c == 1 and j % 2 == 1)
                        eng = nc.gpsimd if on_pool else nc.vector
                        eng.tensor_scalar(
                            ab[:, :],
                            xT_sb[0:CHUNK, c * W + j + 1 : c * W + j + 1 + FD],
                            xTj_sb[0:CHUNK, c * JPC + j : c * JPC + j + 1],
                            0.0,
                            mybir.AluOpType.subtract,
                            mybir.AluOpType.max,
                        )
                    ob = ones_sb[:, 32:64] if (c == 3 and abs3) else ones_sb[:, 0:32]
                    nc.tensor.matmul(
                        dist[32 * c : 32 * c + 32, jj * FD : (jj + 1) * FD],
                        ob,
                        ab[:, :],
                        start=True,
                        stop=(hh % 2 == 0),
                        tile_position=(0, 32 * c),
                        skip_group_check=True,
                    )
                if hh % 2 == 1:
                    # negI path: psum -= S16h[., j+1 : j+257] (closes groups)
                    nc.tensor.matmul(
                        dist[:, jj * FD : (jj + 1) * FD],
                        negI_sb[:, :],
                        S16h_sb[:, j + 1 : j + 1 + FD],
                        start=False,
                        stop=True,
                        skip_group_check=True,
                    )
                if jj == HALF - 1:
                    # batched P = exp(-2*psum) over a 4-row half-group
                    nc.scalar.activation(
                        p8_bufs[hh % NP8][:, :],
                        dist[:, :],
                        mybir.ActivationFunctionType.Exp,
                        bias=0.0,
                        scale=-2.0,
                    )
                    if hh % 2 == 0:
                        # E path: one DVE 2x multiply applies exp(S_b) to the
                        # whole half via an overlapping-window AP over E
                        ewin = EB_sb[:, HALF * hh + 1 : HALF * hh + 1 + FD].copy()
                        ewin.ap = bass_rust.VecI64Pair(
                            [tuple(ewin.ap[0]), (1, HALF), (1, FD)]
                        )
                        nc.vector.tensor_tensor(
                            pe4_bufs[hh % NP8][:, :].rearrange(
                                "p (r c) -> p r c", r=HALF
                            ),
                            p8_bufs[hh % NP8][:, :].rearrange(
                                "p (r c) -> p r c", r=HALF
                            ),
                            ewin,
                            mybir.AluOpType.mult,
                        )

            def emit_rider(j):
                hg = j // HALF
                jj = j % HALF
                src = p8_bufs if hg % 2 == 1 else pe4_bufs
                p8 = src[hg % NP8]
                g_src = G_sb if hg % 2 == 1 else GB_sb
                # dump = P * exp(-S_a);  accum_out = row sum (free on 4x)
                nc.vector.tensor_scalar(
                    dump_bufs[j % NDP][:, :],
                    p8[:, jj * FD : (jj + 1) * FD],
                    g_src[:, j : j + 1],
                    0.0,
                    mybir.AluOpType.mult,
                    mybir.AluOpType.add,
                    accum_out=raw_sb[:, j : j + 1],
                )

            def emit_cross(j):
                # cross[k, j+delta] += dump[k, delta-1] for delta = 1..255
                # (delta=256 belongs to the partner row's own sum)
                nc.gpsimd.tensor_tensor(
                    cross_sb[:, j + 1 : j + FD],
                    cross_sb[:, j + 1 : j + FD],
                    dump_bufs[j % NDP][:, 0 : FD - 1],
                    mybir.AluOpType.add,
                )

            for j in range(-GROUP, JPC + LAG + 1):
                ja = j + GROUP
                if 0 <= ja < JPC and is_ehalf(ja // HALF):
                    emit_abs3(ja)
                if j < 0:
                    continue
                if j < JPC:
                    emit_front(j)
                if LAG <= j < JPC + LAG:
                    emit_rider(j - LAG)
                if j > LAG:
                    emit_cross(j - LAG - 1)
                if j - LAG == 47:
                    nc.scalar.dma_start(
                        out=raw_out[:, 0:48], in_=raw_sb[:, 0:48]
                    )

            psum_es.close()
            nc.scalar.dma_start(out=raw_out[:, 48:JPC], in_=raw_sb[:, 48:JPC])
            nc.sync.dma_start(out=cross_out[:, :], in_=cross_sb[:, :])

    nc.finalize()
    return nc


def _aux_consts():
    # cols 0:32 = 1.0 d-sum pattern; cols 32:64 = 0.5 pattern for ACT Abs
    # chunks (psum holds |z|/2 so exp(-2*psum) = exp(-|z|))
    ob = np.zeros([CHUNK, 64], dtype=np.float16)
    for m in range(KPC):
        ob[5 * m : 5 * m + 5, m] = 1.0
        ob[5 * m : 5 * m + 5, 32 + m] = 0.5
    negI = (-np.eye(128)).astype(np.float16)
    return ob, negI


def make_in_maps(inputs, T):
    f16 = np.float16
    Tm = np.asarray(T, dtype=np.float32).astype(f16)
    ob, negI = _aux_consts()
    in_maps = []
    for c in range(NCORES):
        rolled = np.roll(np.asarray(inputs, dtype=np.float32), -JPC * c, axis=0)
        inTc = np.ascontiguousarray(rolled[0:W].T).astype(f16)
        in_maps.append(
            {
                "inT": inTc,
                "Tm": Tm,
                "onesd": ob,
                "negI": negI,
            }
        )
    return in_maps


def assemble_output(results):
    out = np.zeros([B, K], dtype=np.float32)
    for c in range(NCORES):
        rawc = np.asarray(results[c]["raw"], dtype=np.float32)  # [128, JPC]
        cross = np.asarray(results[c]["cross"], dtype=np.float32)  # [128, W]
        for cc in range(NCHUNK):
            ksl = slice(32 * cc, 32 * cc + KPC)
            kg = slice(KPC * cc, KPC * (cc + 1))
            # own rows: global rows 64c..64c+63 (+1.0 self term)
            out[JPC * c : JPC * (c + 1), kg] += rawc[ksl, :].T + 1.0
            # cross rows: global rows (64c + t) % 512 for t = 1..W-1
            rows = (JPC * c + np.arange(1, W)) % B
            np.add.at(
                out,
                (rows[:, None], np.arange(KPC * cc, KPC * (cc + 1))[None, :]),
                cross[ksl, 1:W].T,
            )
    return out


def kernel(inputs, T):
    from concourse.bass_utils import run_bass_kernel_spmd

    if "nc" not in _NC_CACHE:
        _NC_CACHE["nc"] = build_nc()
    nc = _NC_CACHE["nc"]
    in_maps = make_in_maps(inputs, T)
    res = run_bass_kernel_spmd(nc, in_maps, list(range(NCORES)))
    return assemble_output(res.results)


if __name__ == "__main__":
    sys.path.insert(0, "/root/problem")
    from reference import setup_inputs, reference

    inputs = setup_inputs()
    expected = np.asarray(reference(**inputs))
    actual = kernel(**{k: np.asarray(v) for k, v in inputs.items()})
    err = np.abs(actual - expected)
    rel = np.linalg.norm(actual - expected) / np.linalg.norm(expected)
    print(f"max abs err: {err.max():.3e}")
    print(f"Relative error: {rel:.3e}")


# revision 21
# speedup vs baseline: 1.3400x; 1.0048x over previous
"""
MinibatchDiscrimination kernel for 8x TRN2 NeuronCores (Bass/Tile).

Math:  x = inputs @ T  -> [B, K, D] with B=512, K=100, D=5
       out[a,k] = sum_b exp(-sum_d |x[a,k,d]-x[b,k,d]|)

v4 strategy (v2 = 49.8us, v3 experiments showed per-row ACT exp overhead
and batched-multiply chains were the limiters):

  Pair coverage (as v2): core c owns global rows a = 64c+j (j=0..63) and
  window delta = 1..256 (partners b = a+delta mod 512). Deltas 1..255
  cover each unordered pair once; delta=256 pairs appear from both
  endpoints, and each endpoint keeps its own copy in its row sum while
  the cross path scatters only delta=1..255, so no correction columns
  are needed. The self term exp(0)=1 is added on the host.

  Identity: |u-v| = 2*relu(u-v) - u + v  =>  dist = 2R' - S_b + S_a with
  R' = sum_d relu(x_b - x_a), S = sum_d x. PSUM accumulates
  (R' - S_b/2) per row (4 ones-matmuls + one -I matmul with the halved
  S16 window), so one BIAS-FREE activation computes
      P = exp(-2*psum) = exp(S_b - 2R') = exp(S_a - dist)
  batched over FOUR rows at a time ([128,1024] spanning 2 psum banks,
  260ns/row vs v2's 585ns/row exp+accum). P can reach exp(+17) so P8 is
  stored in BF16 (8-bit exponent - no overflow; 0.4% mantissa error is
  well inside tolerance). A single DVE tensor_scalar rider per row then
  applies the row factor and the row sum in one 4x-mode op:
      dump = P * exp(-S_a)   (G scalar, f32),  accum_out = sum(dump).

  Per row j:
    Pool: relu chunk 0 (ready earliest from the projection), 213ns
    DVE : relu chunks 1,2,3 (fp16 4x mode, 127ns each)
    PE  : 4 d-sum matmuls + 1 negI(S16h window) matmul -> psum
    ACT : exp half-group [128,1024] -> P8 bf16 (at rows 3 and 7 of 8)
    DVE : rider ts: dump = P*G, accum_out -> raw32[:, j]   (127ns)
    Pool: cross[k, j+1..j+255] += dump[k, 0:255]           (213ns)
  Riders lag one half-group behind the front so the DVE queue never
  waits on an exp; crosses follow one row behind riders.

  dist psum layout: partition 32c+m holds k=25c+m (m<25); host
  reassembles own rows from raw32 and scatters cross columns t=1..318
  to rows (64c+t) % 512.
"""

import sys
import numpy as np

for _p in ("/opt/trn_rl_repo",):
    if _p not in sys.path:
        sys.path.insert(0, _p)

B = 512
F = 1024
K = 100
D = 5
KD = K * D  # 500
NCORES = 8
JPC = B // NCORES  # 64 output rows per core
NCHUNK = 4  # kd chunks of 125
CHUNK = KD // NCHUNK  # 125
KPC = K // NCHUNK  # 25 k's per chunk
FD = 256  # per-row window: delta = 1..256
W = JPC + FD  # 320 columns of x needed per core
GROUP = 8  # rows per psum wide tile
HALF = GROUP // 2  # rows per exp batch
LAG = 2 * HALF  # rider emission lag (rows)

_NC_CACHE = {}


def build_nc():
    import contextlib

    import concourse.bass as bass
    import concourse.bacc as bacc
    import concourse.mybir as mybir
    from concourse.tile import TileContext

    nc = bacc.Bacc(None, target_bir_lowering=False, debug=True)

    inT = nc.declare_dram_parameter("inT", [F, W], mybir.dt.float16, isOutput=False)
    Tm = nc.declare_dram_parameter("Tm", [F, KD], mybir.dt.float16, isOutput=False)
    onesd = nc.declare_dram_parameter(
        "onesd", [CHUNK, 64], mybir.dt.float16, isOutput=False
    )
    negI = nc.declare_dram_parameter("negI", [128, 128], mybir.dt.float16, isOutput=False)
    raw_out = nc.declare_dram_parameter(
        "raw", [128, JPC], mybir.dt.float32, isOutput=True
    )
    cross_out = nc.declare_dram_parameter(
        "cross", [128, W], mybir.dt.float32, isOutput=True
    )

    with TileContext(nc) as tc:
        with tc.tile_pool(name="persist", bufs=1) as pp:
            T_sb = pp.tile([128, 8 * KD], mybir.dt.float16, name="T_sb")
            inT_sb = pp.tile([128, 8 * W], mybir.dt.float16, name="inT_sb")
            ones_sb = pp.tile([CHUNK, 64], mybir.dt.float16, name="ones_sb")
            negI_sb = pp.tile([128, 128], mybir.dt.float16, name="negI_sb")
            xT_sb = pp.tile([128, NCHUNK * W], mybir.dt.float16, name="xT_sb")
            # f32 upcasts of xT columns 0..JPC (tensor_scalar per-partition
            # scalars must be f32)
            xTj_sb = pp.tile([128, NCHUNK * JPC], mybir.dt.float32, name="xTj_sb")
            S16h_sb = pp.tile([128, W], mybir.dt.float16, name="S16h_sb")
            E_sb = pp.tile([128, W], mybir.dt.bfloat16, name="E_sb")
            EB_sb = pp.tile([128, W], mybir.dt.bfloat16, name="EB_sb")
            G_sb = pp.tile([128, JPC], mybir.dt.float32, name="G_sb")
            GB_sb = pp.tile([128, JPC], mybir.dt.float32, name="GB_sb")
            negx3_sb = pp.tile([CHUNK, JPC], mybir.dt.float32, name="negx3_sb")
            cross_sb = pp.tile([128, W], mybir.dt.float32, name="cross_sb")
            raw_sb = pp.tile([128, JPC], mybir.dt.float32, name="raw_sb")
            NP8 = 6
            p8_bufs = [
                pp.tile([128, HALF * FD], mybir.dt.bfloat16, name=f"p8_{i}")
                for i in range(NP8)
            ]
            pe4_bufs = [
                pp.tile([128, HALF * FD], mybir.dt.bfloat16, name=f"pe4_{i}")
                for i in range(NP8)
            ]
            NAB = 48
            ab_bufs = [
                pp.tile([CHUNK, FD], mybir.dt.float16, name=f"ab{i}")
                for i in range(NAB)
            ]
            NAB3 = 16
            ab3_bufs = [
                pp.tile([CHUNK, FD], mybir.dt.float16, name=f"ab3_{i}")
                for i in range(NAB3)
            ]
            NDP = 8
            dump_bufs = [
                pp.tile([128, FD], mybir.dt.float16, name=f"dp{i}")
                for i in range(NDP)
            ]

            # warm the ACT exp table while DMAs run (table load ~1.3us)
            warm_sb = pp.tile([1, 1], mybir.dt.float32, name="warm_sb")
            nc.vector.memset(warm_sb[:, :], 0.0)
            nc.scalar.activation(
                warm_sb[:, :], warm_sb[:, :], mybir.ActivationFunctionType.Exp
            )
            nc.vector.memset(cross_sb[:, :], 0.0)

            # --- load inputs: T quarters on the SP queue, inT quarters on
            # the ACT queue so descriptor generation overlaps ---
            for h in range(4):
                nc.sync.dma_start(
                    out=T_sb[:, h * 2 * KD : (h + 1) * 2 * KD],
                    in_=Tm[h * 256 : (h + 1) * 256, :].rearrange(
                        "(t p) c -> p t c", t=2
                    ),
                )
                nc.scalar.dma_start(
                    out=inT_sb[:, h * 2 * W : (h + 1) * 2 * W],
                    in_=inT[h * 256 : (h + 1) * 256, :].rearrange(
                        "(t p) c -> p t c", t=2
                    ),
                )
            nc.sync.dma_start(out=ones_sb[:, :], in_=onesd[:, :])
            nc.sync.dma_start(out=negI_sb[:, :], in_=negI[:, :])

            psum_es = contextlib.ExitStack()
            psum = psum_es.enter_context(
                tc.tile_pool(name="psum", bufs=1, space="PSUM")
            )
            # Four 2-bank-wide dist tiles (all 8 psum banks), one per
            # half-group so WAR tracking decouples each exp from the next
            # halves' matmuls. The projection aliases chunk c's accumulator
            # into wide[c]; S goes into wide[3]'s second bank.
            wide = [
                psum.tile([128, HALF * FD], mybir.dt.float32, name=f"wide{i}")
                for i in range(4)
            ]
            xt_ps = [wide[c][0:CHUNK, 0:W] for c in range(NCHUNK)]
            S_ps = wide[3][:, 512 : 512 + W]
            # projection: t-outer for tiles 0..5 (runnable as DMA quarters
            # land), then per-chunk tails so each chunk's psum->sbuf copy
            # starts while the next chunk's tail matmuls run
            for t in range(6):
                for c in range(NCHUNK):
                    nc.tensor.matmul(
                        xt_ps[c],
                        T_sb[:, t * KD + c * CHUNK : t * KD + (c + 1) * CHUNK],
                        inT_sb[:, t * W : (t + 1) * W],
                        start=(t == 0),
                        stop=False,
                        skip_group_check=True,
                    )
            for c in range(NCHUNK):
                for t in (6, 7):
                    nc.tensor.matmul(
                        xt_ps[c],
                        T_sb[:, t * KD + c * CHUNK : t * KD + (c + 1) * CHUNK],
                        inT_sb[:, t * W : (t + 1) * W],
                        start=False,
                        stop=(t == 7),
                        skip_group_check=True,
                    )
                # psum->sbuf copies split between DVE and ACT
                eng = nc.vector.tensor_copy if c % 2 == 0 else nc.scalar.copy
                eng(xT_sb[0:CHUNK, c * W : (c + 1) * W], xt_ps[c])
                if c % 2 == 0:
                    nc.vector.tensor_copy(
                        xTj_sb[0:CHUNK, c * JPC : (c + 1) * JPC],
                        xT_sb[0:CHUNK, c * W : c * W + JPC],
                    )
                else:
                    nc.scalar.copy(
                        xTj_sb[0:CHUNK, c * JPC : (c + 1) * JPC],
                        xT_sb[0:CHUNK, c * W : c * W + JPC],
                    )
                # S[25c+m at partition 32c+m, i] = sum_d x[kd, i]
                nc.tensor.matmul(
                    S_ps[32 * c : 32 * c + 32, :],
                    ones_sb[:, 0:32],
                    xT_sb[0:CHUNK, c * W : (c + 1) * W],
                    start=True,
                    stop=True,
                    tile_position=(0, 32 * c),
                )
            # S16h = S/2 (negI matmul's moving operand: psum gets -S_b/2 so
            # exp(-2*psum) carries exp(+S_b)); G = exp(-S) f32 row scalars
            nc.scalar.mul(S16h_sb[:, :], S_ps[:, :], 0.5)
            nc.scalar.activation(
                E_sb[:, :], S_ps[:, :], mybir.ActivationFunctionType.Exp,
                bias=0.0, scale=1.0,
            )
            nc.scalar.activation(
                G_sb[:, :], S_ps[:, 0:JPC], mybir.ActivationFunctionType.Exp,
                bias=0.0, scale=-1.0,
            )
            # B variants for ACT-Abs halves: chunk-3 k's (partitions 96+)
            # need no S correction (their |z| lands exactly in psum)
            nc.vector.tensor_scalar(
                negx3_sb[:, :], xTj_sb[0:CHUNK, 3 * JPC : 4 * JPC], -1.0, 0.0,
                mybir.AluOpType.mult, mybir.AluOpType.add,
            )
            nc.vector.tensor_copy(EB_sb[:, :], E_sb[:, :])
            nc.vector.memset(EB_sb[96:128, :], 1.0)
            nc.vector.tensor_copy(GB_sb[:, :], G_sb[:, :])
            nc.vector.memset(GB_sb[96:128, :], 1.0)

            import bass_rust

            def is_ehalf(hh):
                return hh % 2 == 0

            def emit_abs3(j):
                # ACT absorbs chunk 3 of E-half rows: |x_win - x_j| written
                # a group ahead so PE never waits on the ACT queue
                nc.scalar.activation(
                    ab3_bufs[j % NAB3][:, :],
                    xT_sb[0:CHUNK, 3 * W + j + 1 : 3 * W + j + 1 + FD],
                    mybir.ActivationFunctionType.Abs,
                    bias=negx3_sb[:, j : j + 1],
                    scale=1.0,
                )

            # --- main loop over output rows ---
            def emit_front(j):
                hh = j // HALF
                jj = j % HALF
                dist = wide[hh % 4]
                abs3 = is_ehalf(hh)
                for c in range(NCHUNK):
                    if c == 3 and abs3:
                        ab = ab3_bufs[j % NAB3]
                    else:
                        ab = ab_bufs[(j * NCHUNK + c) % NAB]
                        # Pool takes chunk 0 (ready earliest) plus chunk 1 on
                        # every other row; DVE the rest
                        on_pool = c == 0 or (c == 1 and j % 4 == 1)
                        eng = nc.gpsimd if on_pool else nc.vector
                        eng.tensor_scalar(
                            ab[:, :],
                            xT_sb[0:CHUNK, c * W + j + 1 : c * W + j + 1 + FD],
                            xTj_sb[0:CHUNK, c * JPC + j : c * JPC + j + 1],
                            0.0,
                            mybir.AluOpType.subtract,
                            mybir.AluOpType.max,
                        )
                    ob = ones_sb[:, 32:64] if (c == 3 and abs3) else ones_sb[:, 0:32]
                    nc.tensor.matmul(
                        dist[32 * c : 32 * c + 32, jj * FD : (jj + 1) * FD],
                        ob,
                        ab[:, :],
                        start=True,
                        stop=(hh % 2 == 0),
                        tile_position=(0, 32 * c),
                        skip_group_check=True,
                    )
                if hh % 2 == 1:
                    # negI path: psum -= S16h[., j+1 : j+257] (closes groups)
                    nc.tensor.matmul(
                        dist[:, jj * FD : (jj + 1) * FD],
                        negI_sb[:, :],
                        S16h_sb[:, j + 1 : j + 1 + FD],
                        start=False,
                        stop=True,
                        skip_group_check=True,
                    )
                if jj == HALF - 1:
                    # batched P = exp(-2*psum) over a 4-row half-group
                    nc.scalar.activation(
                        p8_bufs[hh % NP8][:, :],
                        dist[:, :],
                        mybir.ActivationFunctionType.Exp,
                        bias=0.0,
                        scale=-2.0,
                    )
                    if hh % 2 == 0:
                        # E path: one DVE 2x multiply applies exp(S_b) to the
                        # whole half via an overlapping-window AP over E
                        ewin = EB_sb[:, HALF * hh + 1 : HALF * hh + 1 + FD].copy()
                        ewin.ap = bass_rust.VecI64Pair(
                            [tuple(ewin.ap[0]), (1, HALF), (1, FD)]
                        )
                        nc.vector.tensor_tensor(
                            pe4_bufs[hh % NP8][:, :].rearrange(
                                "p (r c) -> p r c", r=HALF
                            ),
                            p8_bufs[hh % NP8][:, :].rearrange(
                                "p (r c) -> p r c", r=HALF
                            ),
                            ewin,
                            mybir.AluOpType.mult,
                        )

            def emit_rider(j):
                hg = j // HALF
                jj = j % HALF
                src = p8_bufs if hg % 2 == 1 else pe4_bufs
                p8 = src[hg % NP8]
                g_src = G_sb if hg % 2 == 1 else GB_sb
                # dump = P * exp(-S_a);  accum_out = row sum (free on 4x)
                nc.vector.tensor_scalar(
                    dump_bufs[j % NDP][:, :],
                    p8[:, jj * FD : (jj + 1) * FD],
                    g_src[:, j : j + 1],
                    0.0,
                    mybir.AluOpType.mult,
                    mybir.AluOpType.add,
                    accum_out=raw_sb[:, j : j + 1],
                )

            def emit_cross(j):
                # cross[k, j+delta] += dump[k, delta-1] for delta = 1..255
                # (delta=256 belongs to the partner row's own sum)
                nc.gpsimd.tensor_tensor(
                    cross_sb[:, j + 1 : j + FD],
                    cross_sb[:, j + 1 : j + FD],
                    dump_bufs[j % NDP][:, 0 : FD - 1],
                    mybir.AluOpType.add,
                )

            for j in range(-GROUP, JPC + LAG + 1):
                ja = j + GROUP
                if 0 <= ja < JPC and is_ehalf(ja // HALF):
                    emit_abs3(ja)
                if j < 0:
                    continue
                if j < JPC:
                    emit_front(j)
                if LAG <= j < JPC + LAG:
                    emit_rider(j - LAG)
                if j > LAG:
                    emit_cross(j - LAG - 1)
                if j - LAG == 47:
                    nc.scalar.dma_start(
                        out=raw_out[:, 0:48], in_=raw_sb[:, 0:48]
                    )

            psum_es.close()
            nc.scalar.dma_start(out=raw_out[:, 48:JPC], in_=raw_sb[:, 48:JPC])
            nc.sync.dma_start(out=cross_out[:, :], in_=cross_sb[:, :])

    nc.finalize()
    return nc


def _aux_consts():
    # cols 0:32 = 1.0 d-sum pattern; cols 32:64 = 0.5 pattern for ACT Abs
    # chunks (psum holds |z|/2 so exp(-2*psum) = exp(-|z|))
    ob = np.zeros([CHUNK, 64], dtype=np.float16)
    for m in range(KPC):
        ob[5 * m : 5 * m + 5, m] = 1.0
        ob[5 * m : 5 * m + 5, 32 + m] = 0.5
    negI = (-np.eye(128)).astype(np.float16)
    return ob, negI


def make_in_maps(inputs, T):
    f16 = np.float16
    Tm = np.asarray(T, dtype=np.float32).astype(f16)
    ob, negI = _aux_consts()
    in_maps = []
    for c in range(NCORES):
        rolled = np.roll(np.asarray(inputs, dtype=np.float32), -JPC * c, axis=0)
        inTc = np.ascontiguousarray(rolled[0:W].T).astype(f16)
        in_maps.append(
            {
                "inT": inTc,
                "Tm": Tm,
                "onesd": ob,
                "negI": negI,
            }
        )
    return in_maps


def assemble_output(results):
    out = np.zeros([B, K], dtype=np.float32)
    for c in range(NCORES):
        rawc = np.asarray(results[c]["raw"], dtype=np.float32)  # [128, JPC]
        cross = np.asarray(results[c]["cross"], dtype=np.float32)  # [128, W]
        for cc in range(NCHUNK):
            ksl = slice(32 * cc, 32 * cc + KPC)
            kg = slice(KPC * cc, KPC * (cc + 1))
            # own rows: global rows 64c..64c+63 (+1.0 self term)
            out[JPC * c : JPC * (c + 1), kg] += rawc[ksl, :].T + 1.0
            # cross rows: global rows (64c + t) % 512 for t = 1..W-1
            rows = (JPC * c + np.arange(1, W)) % B
            np.add.at(
                out,
                (rows[:, None], np.arange(KPC * cc, KPC * (cc + 1))[None, :]),
                cross[ksl, 1:W].T,
            )
    return out


def kernel(inputs, T):
    from concourse.bass_utils import run_bass_kernel_spmd

    if "nc" not in _NC_CACHE:
        _NC_CACHE["nc"] = build_nc()
    nc = _NC_CACHE["nc"]
    in_maps = make_in_maps(inputs, T)
    res = run_bass_kernel_spmd(nc, in_maps, list(range(NCORES)))
    return assemble_output(res.results)


if __name__ == "__main__":
    sys.path.insert(0, "/root/problem")
    from reference import setup_inputs, reference

    inputs = setup_inputs()
    expected = np.asarray(reference(**inputs))
    actual = kernel(**{k: np.asarray(v) for k, v in inputs.items()})
    err = np.abs(actual - expected)
    rel = np.linalg.norm(actual - expected) / np.linalg.norm(expected)
    print(f"max abs err: {err.max():.3e}")
    print(f"Relative error: {rel:.3e}")
